# revision 60
# baseline (speedup 1.0000x reference)
"""Trainium2 Bass kernel for nn_DeltaNet_22488448762128 (v3).

Full-input contract: kernel(**inputs) takes the unsharded numpy inputs and
returns the full [B, L, HID] output. Internally shards across 8 NeuronCores:
core = (b, hg) with b in {0,1} and hg in {0..3} head-groups of 4 heads.
Each core computes projections for its 4 heads, a chunked (C=128) linear
attention scan, and a partial output projection; the host sums the 4 partial
outputs per batch element and adds bo.

Math (per head, chunk c of size C, state S aug with z column):
  a_t   = cumprod(beta) within chunk;  aC = a_{C-1}
  q~_t  = phi(rope(q))_t * a_t ;  k^_s = phi(rope(k))_s * aC / a_s
  A^T[s,t] = (phi_k_s . q~_t) * (1/a_s) * [s<=t]
  nu    = A^T.T @ [V|1] + q~ @ S_aug   ;  y_t = nu[:, :D] / (nu[:, D] + eps)
  S_aug = aC * S_aug + k^T @ [V|1]

Precision: projections and the output projection run on the PE in fp8 (e4m3)
DoubleRow mode (pairs two 128-row contraction tiles per pass) with
error-compensated term stacks
  x @ W  =  x8 @ W8  [+ xl8 @ W8']  [+ x8 @ Wl8]
(x8 = fp8(x), xl8 = fp8(16*(x - x8)), Wl = W - fp8(32W)/32). Measured
per-path sensitivity sets the term counts (WTERMS): q/k run 1-term (their
quantization noise cancels through the normalized attention), v and the
output projection need all terms (their noise reaches y linearly), g is
free. Weights are pre-scaled by 32 into e4m3's normal range; the inverse
scale is folded into the rope tables, the sigmoid's activation scale, the
augmented-ones column (=32), and the final output-copy scale, so unscaling
costs zero extra instructions. The reference's +eps (1e-6) on the strictly
positive denominator is dropped. The scan runs in bf16 operands with fp32
PSUM accumulation; the rope/phi elementwise path is bf16 end-to-end (2-4x
DVE modes). Output is returned bf16; the host sums partials in fp32.

PSUM banks (8): pj[big x2] g/q/k/v projection groups (+ final-po overflow);
ppo[po x1] output projection; ptp[tp6 x2] per-head-PAIR q/k/y transposes
(one wide 2x-mode DVE copy per pair instead of per-head ACT copies);
pnu[nuA x2] per-head A|nu|U groups (A's group is closed before the nu/U
group reopens the bank; the reopening matmul consumes A_sb, so it naturally
orders after the DVE mask's Ar readback); pbeta[bt x1] the beta-chain
transposes, decoupled from the scan's pnu ring so the two-chunk-ahead
projection phase can't stall the scan.

Schedule: two-level software pipeline. Outer: proj_phase(c+2) (projections,
beta chain, rope/phi, khat) is emitted two chunks ahead of scan_phase(c),
so the long rope->phi elementwise chains overlap earlier chunks' scans and
the PE never waits on them in steady state. Inner: 2-wide pipelined head
pairs; the previous chunk's output projection is interleaved as PE filler
(o-tiles 0-2 at the first pair, 3 at the second); the final chunk's po
alternates ppo/pj banks so its groups overlap their readbacks. The rope
tables are fetched once (not per chunk); small constants are deferred past
wq on the serialized HWDGE so chunk 0's projections start ~1.5us earlier.
GPSIMD (no PSUM access, no stt opcode) carries SBUF-local q-side work:
rope(q)'s second rotation half, phi(q)'s min and its relu+add assembly.
The y scale (1/denom) is applied on ScalarE (activation Copy with a
per-partition scale pointer), freeing DVE for the PSUM-bound scan ops.
"""

import math
import numpy as np
import ml_dtypes

B, L, HID = 2, 2048, 2048
H, D = 16, 128
HG = 4              # heads per core
C = 128             # chunk size
NCHUNK = L // C     # 16
NK = HID // C       # 16 contraction tiles
NKP = NK // 2       # 8 DoubleRow pair-tiles
EPS = 1e-6
BETA_MIN, BETA_MAX = 0.8, 0.9995
NCORES = 8
GW = HG * D         # 512, per-core projection width
NO = HID // GW      # 4 output col tiles
SCALE = 32.0        # fp8 weight pre-scale
XL_S = 16.0         # x residual pre-scale
E4 = ml_dtypes.float8_e4m3

# fp8 GEMM term counts: 2 = x8@W8 + xl8@W8' (x-quantization compensated),
# 3 = + x8@Wl8 (weight-quantization compensated too). g is ~free (N=4) so it
# keeps 3 terms; q/k/v tolerate W-quant noise (it largely cancels or averages
# out through the normalized attention).
WTERMS = {"q": 1, "k": 1, "v": 3, "g": 3}
PO_SLOTS = ((0, 0), (0, 1), (1, 0))  # (wo term slot, y source: 0=y8 1=ylow)

_CACHE = {}


def _rope_tables():
    half = D // 2
    inv_freq = (1.0 / (10000.0 ** (np.arange(half, dtype=np.float32) /
                                   np.float32(half)))).astype(np.float32)
    t = np.arange(L, dtype=np.float32)
    freqs = t[:, None] * inv_freq[None, :]
    # fold the fp8 weight pre-scale out of q/k here: tables are cos/32, sin/32
    cos = (np.cos(freqs) / SCALE).astype(ml_dtypes.bfloat16)   # [L, 64]
    sin = (np.sin(freqs) / SCALE).astype(ml_dtypes.bfloat16)
    # chunk-major: [128, NCHUNK*64], block c = rows c*128..c*128+128
    def rearr(m):
        return np.ascontiguousarray(
            m.reshape(NCHUNK, C, half).transpose(1, 0, 2).reshape(C, NCHUNK * half))
    return rearr(cos), rearr(sin)


def _build(cfg):
    import concourse.bass as bass
    import concourse.bacc as bacc
    import concourse.tile as tile
    import concourse.mybir as mybir
    from contextlib import ExitStack

    dt = mybir.dt
    F32 = dt.float32
    BF16 = dt.bfloat16
    F8 = dt.float8e4
    DRm = mybir.MatmulPerfMode.DoubleRow
    Alu = mybir.AluOpType
    Act = mybir.ActivationFunctionType
    half = D // 2

    nch = cfg.get("nchunk", NCHUNK)

    nc = bacc.Bacc("TRN2", target_bir_lowering=False, debug=False,
                   enable_asserts=False, num_devices=NCORES)

    # ---- DRAM I/O (host passes PE-blocked layouts, see make_in_maps) ----
    xT_d = nc.dram_tensor("xTb", [NCHUNK, C, 2 * HID], F8, kind="ExternalInput").ap()
    wq_d = nc.dram_tensor("wq3", [C, WTERMS["q"] * NK * GW], F8,
                          kind="ExternalInput").ap()
    wk_d = nc.dram_tensor("wk3", [C, WTERMS["k"] * NK * GW], F8,
                          kind="ExternalInput").ap()
    wv_d = nc.dram_tensor("wv3", [C, WTERMS["v"] * NK * GW], F8,
                          kind="ExternalInput").ap()
    wg_d = nc.dram_tensor("wg3", [C, WTERMS["g"] * NK * HG], F8,
                          kind="ExternalInput").ap()
    wo_d = nc.dram_tensor("wo3", [C, 2 * HG * HID], F8, kind="ExternalInput").ap()
    nbg_d = nc.dram_tensor("nbg4", [C, HG], F32, kind="ExternalInput").ap()
    cos_d = nc.dram_tensor("cosr", [C, NCHUNK * half], BF16, kind="ExternalInput").ap()
    sin_d = nc.dram_tensor("sinr", [C, NCHUNK * half], BF16, kind="ExternalInput").ap()
    mask_d = nc.dram_tensor("maskT", [C, C], F32, kind="ExternalInput").ap()
    id_d = nc.dram_tensor("ident", [C, C], F32, kind="ExternalInput").ap()
    out_d = nc.dram_tensor("out", [L, HID], BF16, kind="ExternalOutput").ap()

    def pair(t, off, step, f):
        b = t[:]
        return bass.AP(tensor=b.tensor, offset=b.offset + off,
                       ap=[b.ap[0], [step, 2], [1, f]])

    with ExitStack() as ctx:
        tc = ctx.enter_context(tile.TileContext(nc))

        cpool = ctx.enter_context(tc.tile_pool(name="consts", bufs=1))
        mask_t = cpool.tile([C, C], F32, tag="mask")
        id_t = cpool.tile([C, C], F32, tag="id")
        id_s = cpool.tile([C, C], BF16, tag="id_s")
        ones_t = cpool.tile([C, C], F32, tag="ones")
        nbg_t = cpool.tile([C, HG], F32, tag="nbg")
        nc.vector.memset(ones_t[:], 1.0)

        with ExitStack() as main:
            wpool = main.enter_context(tc.tile_pool(name="w", bufs=1))
            wq_t = wpool.tile([C, WTERMS["q"] * NK * GW], F8, tag="wq")
            wk_t = wpool.tile([C, WTERMS["k"] * NK * GW], F8, tag="wk")
            wv_t = wpool.tile([C, WTERMS["v"] * NK * GW], F8, tag="wv")
            wg_t = wpool.tile([C, WTERMS["g"] * NK * HG], F8, tag="wg")
            wo_t = wpool.tile([C, 2 * HG * HID], F8, tag="wo")
            nc.sync.dma_start(wg_t[:], wg_d)

            # chunk-local SBUF pools
            xp = main.enter_context(tc.tile_pool(name="xp", bufs=cfg.get("xp", 3)))
            cspool = main.enter_context(tc.tile_pool(name="cspool", bufs=cfg.get("csp", 6)))

            def fetch_x(t, ci, eng=None):
                eng = eng or nc.scalar
                eng.dma_start(t[:, 0:HID], xT_d[ci][:, 0:HID])
                eng.dma_start(t[:, HID:2 * HID], xT_d[ci][:, HID:2 * HID])

            # whole-kernel rope tables fetched once (2 DMAs instead of 32):
            # per-chunk cos/sin are views into these resident tiles
            costab = cspool.tile([C, NCHUNK * half], BF16, tag="costab", bufs=1)
            sintab = cspool.tile([C, NCHUNK * half], BF16, tag="sintab", bufs=1)

            def fetch_cs(ci, eng=None):
                return (costab[:, bass.ts(ci, half)],
                        sintab[:, bass.ts(ci, half)])

            # prefetch chunk 0/1 x + rope tables ahead of the weight
            # stream so chunk 0 isn't queued behind 9 MB of weights
            xpre = []
            cspre = []
            for cpre in range(min(2, nch)):
                t = xp.tile([C, 2 * HID], F8, tag="xtb")
                xpre.append(t)
            fetch_x(xpre[0], 0)
            cspre.append(fetch_cs(0))

            # weights streamed in PE consumption order (g,q,k,v then wo),
            # sliced so the PE can trail the DMA k-pair by k-pair; small
            # constants deferred past wq so chunk 0's q projection isn't
            # queued behind them on the serialized HWDGE
            TW = NK * GW

            def wslices(w_t, w_d, nt):
                for term in range(nt):
                    for hf in range(2):
                        sl = slice(term * TW + hf * TW // 2,
                                   term * TW + (hf + 1) * TW // 2)
                        nc.sync.dma_start(w_t[:, sl], w_d[:, sl])

            wslices(wq_t, wq_d, WTERMS["q"])
            nc.scalar.dma_start(costab[:], cos_d)
            nc.scalar.dma_start(sintab[:], sin_d)
            nc.sync.dma_start(id_t[:], id_d)
            nc.sync.dma_start(nbg_t[:], nbg_d)
            nc.scalar.copy(id_s[:], id_t[:])
            cspre.append(fetch_cs(1))
            wslices(wk_t, wk_d, WTERMS["k"])
            fetch_x(xpre[1], 1)
            nc.sync.dma_start(mask_t[:], mask_d)
            wslices(wv_t, wv_d, WTERMS["v"])
            for term in (0, 1):
                ts_ = bass.ts(term, HG * HID)
                nc.sync.dma_start(wo_t[:, ts_], wo_d[:, ts_])
            big2 = main.enter_context(tc.tile_pool(name="big2", bufs=cfg.get("big2", 2)))
            sml = main.enter_context(tc.tile_pool(name="sml", bufs=cfg.get("sml", 4)))
            spool = main.enter_context(tc.tile_pool(name="spool", bufs=2))
            ypool = main.enter_context(tc.tile_pool(name="ypool", bufs=2))
            osb = main.enter_context(tc.tile_pool(name="osb", bufs=cfg.get("osb", 5)))

            # psum pools: pj 2 + ppo 2 + ptp 2 + pnu 2 = 8 banks
            pj = main.enter_context(tc.tile_pool(
                name="pj", bufs=cfg.get("pj", 2), space="PSUM"))
            ppo = main.enter_context(tc.tile_pool(
                name="ppo", bufs=cfg.get("ppo", 1), space="PSUM"))
            ptp = main.enter_context(tc.tile_pool(
                name="ptp", bufs=cfg.get("ptp", 2), space="PSUM"))
            pnu = main.enter_context(tc.tile_pool(
                name="pnu", bufs=cfg.get("pnu", 2), space="PSUM"))
            pbeta = main.enter_context(tc.tile_pool(
                name="pbeta", bufs=cfg.get("pbeta", 1), space="PSUM"))

            S_cur = []
            for h in range(HG):
                s0 = spool.tile([C, D + 1], BF16, tag=f"s{h}")
                nc.vector.memset(s0[:], 0.0)
                S_cur.append(s0)

            def proj_mms(ps, fw, w_t, tw, nterm, xtb):
                n = 0
                for term in range(nterm):
                    xoff = HID if term == 1 else 0
                    for kp in range(NKP):
                        nc.tensor.matmul(
                            ps[:, 0:fw],
                            pair(xtb, xoff + kp * 2 * C, C, C),
                            pair(w_t, term * tw + kp * 2 * fw, fw, fw),
                            start=(n == 0), stop=(n == nterm * NKP - 1),
                            perf_mode=DRm)
                        n += 1

            def rope(src, dst, tmp, cs, tmp2=None):
                # tmp2 set: de-half on DVE, do-half on GPSIMD concurrently
                cos_c, sin_c = cs
                ed = nc.vector
                eo = nc.gpsimd if tmp2 is not None else nc.vector
                se = src[:].rearrange("p (h d) -> p h d", h=HG)[:, :, 0:half]
                so = src[:].rearrange("p (h d) -> p h d", h=HG)[:, :, half:D]
                de = dst[:].rearrange("p (h d) -> p h d", h=HG)[:, :, 0:half]
                do = dst[:].rearrange("p (h d) -> p h d", h=HG)[:, :, half:D]
                cc = bass.AP(tensor=cos_c.tensor, offset=cos_c.offset,
                             ap=[cos_c.ap[0], [0, HG], [1, half]])
                ss = bass.AP(tensor=sin_c.tensor, offset=sin_c.offset,
                             ap=[sin_c.ap[0], [0, HG], [1, half]])
                t1 = tmp[:].rearrange("p (h d) -> p h d", h=HG)[:, :, 0:half]
                t2 = tmp[:].rearrange("p (h d) -> p h d", h=HG)[:, :, half:D]
                tb = tmp2 if tmp2 is not None else tmp
                t3 = tb[:].rearrange("p (h d) -> p h d", h=HG)[:, :, 0:half]
                t4 = tb[:].rearrange("p (h d) -> p h d", h=HG)[:, :, half:D]
                ed.tensor_tensor(out=t1, in0=se, in1=cc, op=Alu.mult)
                ed.tensor_tensor(out=t2, in0=so, in1=ss, op=Alu.mult)
                ed.tensor_tensor(out=de, in0=t1, in1=t2, op=Alu.subtract)
                eo.tensor_tensor(out=t3, in0=se, in1=ss, op=Alu.mult)
                eo.tensor_tensor(out=t4, in0=so, in1=cc, op=Alu.mult)
                eo.tensor_tensor(out=do, in0=t3, in1=t4, op=Alu.add)

            po_prev = None  # (yt8, ytl) of previous chunk

            def emit_po(ysrcs, c_out, orange=None, final=False):
                yt8_, ytl_ = ysrcs
                nmm = 2 * len(PO_SLOTS)
                for o in (orange if orange is not None else range(NO)):
                    # in the epilogue the projection banks are idle: alternate
                    # pools so o-tile groups overlap their readback copies
                    if final and o % 2 == 1:
                        out_ps = pj.tile([C, GW], F32, tag="big")
                    else:
                        out_ps = ppo.tile([C, GW], F32, tag="po")
                    n = 0
                    # hp-outer: the first MMs only need heads 0-1, so the
                    # group can start before heads 2-3 finish
                    for hp in range(HG // 2):
                        for slot, ysel in PO_SLOTS:
                            ysrc = yt8_ if ysel == 0 else ytl_
                            nc.tensor.matmul(
                                out_ps[:],
                                pair(ysrc, hp * 2 * C, C, C),
                                pair(wo_t,
                                     slot * HG * HID + (2 * hp) * HID + o * GW,
                                     HID, GW),
                                start=(n == 0), stop=(n == nmm - 1),
                                perf_mode=DRm)
                            n += 1
                    out_sb = osb.tile([C, GW], BF16, tag="osb")
                    if o % 2 == 0:
                        nc.scalar.mul(out_sb[:], out_ps[:], 1.0 / SCALE)
                    else:
                        nc.vector.tensor_scalar_mul(out_sb[:], out_ps[:],
                                                    1.0 / SCALE)
                    nc.sync.dma_start(out_d[bass.ts(c_out, C), bass.ts(o, GW)],
                                      out_sb[:])

            def proj_phase(c):
                """Projections + beta chain + rope/phi/khat for chunk c.
                Emitted two chunks ahead of the scan so the elementwise
                chains overlap earlier chunks' scans."""
                if c < len(xpre):
                    xtb = xpre[c]
                    cos_c, sin_c = cspre[c]
                else:
                    xtb = xp.tile([C, 2 * HID], F8, tag="xtb")
                    fetch_x(xtb, c)
                    cos_c, sin_c = fetch_cs(c)

                # ---- projections (PE) interleaved with beta chain ----
                # g first so the long beta dependency chain starts early
                g_ps = pj.tile([C, GW], F32, tag="big")
                proj_mms(g_ps, HG, wg_t, NK * HG, WTERMS["g"], xtb)
                beta_sb = sml.tile([C, HG], F32, tag="beta")
                nc.scalar.activation(beta_sb[:], g_ps[:, 0:HG], Act.Exp,
                                     scale=-1.0 / SCALE)
                nc.vector.scalar_tensor_tensor(
                    out=beta_sb[:], in0=beta_sb[:], scalar=1.0,
                    in1=nbg_t[:], op0=Alu.mult, op1=Alu.mult)
                nc.vector.tensor_scalar_add(beta_sb[:], beta_sb[:], 1.0)
                nc.vector.reciprocal(beta_sb[:], beta_sb[:])
                nc.vector.tensor_scalar(out=beta_sb[:], in0=beta_sb[:],
                                        scalar1=BETA_MIN, scalar2=BETA_MAX,
                                        op0=Alu.max, op1=Alu.min)

                q_ps = pj.tile([C, GW], F32, tag="big")
                proj_mms(q_ps, GW, wq_t, TW, WTERMS["q"], xtb)
                q_sb = big2.tile([C, GW], BF16, tag="q")
                nc.scalar.copy(q_sb[:], q_ps[:])

                # beta transposes ride the pnu rotation between head uses
                btp_ps = pbeta.tile([C, C], F32, tag="bt")
                nc.tensor.transpose(btp_ps[0:HG, 0:C], beta_sb[:], id_t[:])
                btp_sb = sml.tile([HG, C], F32, tag="btp")
                nc.scalar.copy(btp_sb[:], btp_ps[0:HG, 0:C])
                aT_sb = sml.tile([HG, C], F32, tag="aT")
                nc.vector.tensor_tensor_scan(
                    out=aT_sb[:], data0=btp_sb[:], data1=ones_t[0:HG, :],
                    initial=1.0, op0=Alu.mult, op1=Alu.mult)

                k_ps = pj.tile([C, GW], F32, tag="big")
                proj_mms(k_ps, GW, wk_t, TW, WTERMS["k"], xtb)
                k_sb = big2.tile([C, GW], BF16, tag="k")
                nc.scalar.copy(k_sb[:], k_ps[:])

                a_ps = pbeta.tile([C, C], F32, tag="bt")
                nc.tensor.transpose(a_ps[:, 0:HG], aT_sb[:], id_t[0:HG, 0:HG])
                a_sb = sml.tile([C, HG], F32, tag="a")
                nc.scalar.copy(a_sb[:], a_ps[:, 0:HG])
                ainv_sb = sml.tile([C, HG], F32, tag="ainv")
                nc.vector.reciprocal(ainv_sb[:], a_sb[:])
                diag4 = sml.tile([HG, HG], F32, tag="diag4")
                nc.vector.tensor_scalar(out=diag4[:], in0=id_t[0:HG, 0:HG],
                                        scalar1=aT_sb[:, C - 1:C], scalar2=None,
                                        op0=Alu.mult)

                # rope(q) early on DVE; phi(q) min + assembly on GPSIMD
                # (the q path is off the serial S/z recurrence, so the slow
                # Pool engine can carry it)
                qr = big2.tile([C, GW], BF16, tag="qr")
                rtq = big2.tile([C, GW], BF16, tag="rtq")
                rtq2 = big2.tile([C, GW], BF16, tag="rtq2")
                rope(q_sb, qr, rtq, (cos_c, sin_c), rtq2)
                tmq = big2.tile([C, GW], BF16, tag="mq")
                nc.gpsimd.tensor_scalar_min(tmq[:], qr[:], 0.0)
                teq = big2.tile([C, GW], BF16, tag="eq")
                nc.scalar.activation(teq[:], tmq[:], Act.Exp)
                # phi(q) assembled on GPSIMD (no stt opcode there: relu + add)
                rlq = big2.tile([C, GW], BF16, tag="rlq")
                nc.gpsimd.tensor_scalar_max(rlq[:], qr[:], 0.0)
                phiq = big2.tile([C, GW], BF16, tag="phq", bufs=4)
                nc.gpsimd.tensor_tensor(out=phiq[:], in0=rlq[:], in1=teq[:],
                                        op=Alu.add)

                v_ps = pj.tile([C, GW], F32, tag="big")
                proj_mms(v_ps, GW, wv_t, TW, WTERMS["v"], xtb)
                v_sb = big2.tile([C, HG * (D + 1)], BF16, tag="v", bufs=4)
                v_aug = v_sb[:].rearrange("p (h e) -> p h e", e=D + 1)
                nc.scalar.copy(v_aug[:, :, 0:D],
                               v_ps[:, 0:GW].rearrange("p (h e) -> p h e", e=D))
                nc.vector.memset(v_aug[:, :, D:D + 1], SCALE)

                acb_ps = pbeta.tile([C, C], F32, tag="bt")
                nc.tensor.matmul(acb_ps[:, 0:HG], ones_t[0:HG, :], diag4[:],
                                 start=True, stop=True)
                acb_sb = sml.tile([C, HG], F32, tag="acb")
                nc.scalar.copy(acb_sb[:], acb_ps[:, 0:HG])
                acdiv_sb = sml.tile([C, HG], F32, tag="acdiv")
                nc.vector.tensor_tensor(out=acdiv_sb[:], in0=ainv_sb[:],
                                        in1=acb_sb[:], op=Alu.mult)

                # rope(k) + phi(k) stay on the fast engines: the k-path feeds
                # the serial S/z recurrence (khat -> U -> S_new)
                kr = big2.tile([C, GW], BF16, tag="kr")
                rtk = big2.tile([C, GW], BF16, tag="rtk")
                rope(k_sb, kr, rtk, (cos_c, sin_c))
                tmk = big2.tile([C, GW], BF16, tag="mk")
                nc.vector.tensor_scalar_min(tmk[:], kr[:], 0.0)
                tek = big2.tile([C, GW], BF16, tag="ek")
                nc.scalar.activation(tek[:], tmk[:], Act.Exp)
                phik = big2.tile([C, GW], BF16, tag="phk", bufs=4)
                nc.vector.scalar_tensor_tensor(out=phik[:], in0=kr[:],
                                               scalar=0.0, in1=tek[:],
                                               op0=Alu.max, op1=Alu.add)

                # khat for all heads, hoisted off the per-head critical path
                khats = []
                for h in range(HG):
                    khat = sml.tile([C, D], BF16, tag="khat", bufs=12)
                    nc.vector.tensor_scalar_mul(khat[:], phik[:, bass.ts(h, D)],
                                                acdiv_sb[:, h:h + 1])
                    khats.append(khat[:])

                return dict(phiq=phiq, phik=phik, khats=khats, v_sb=v_sb,
                            ainv_sb=ainv_sb, acb_sb=acb_sb)

            def scan_phase(c, P, last=False):
                nonlocal po_prev
                phiq, phik = P["phiq"], P["phik"]
                khats, v_sb = P["khats"], P["v_sb"]
                ainv_sb, acb_sb = P["ainv_sb"], P["acb_sb"]

                # ---- scan, 2-wide head pipeline ----
                use_ytl = any(ysel == 1 for _, ysel in PO_SLOTS)
                yt8 = ypool.tile([C, HG * C], F8, tag="yt8")
                if use_ytl:
                    ytl = ypool.tile([C, HG * C], F8, tag="ytl")
                else:
                    ytl = None

                def pair_tp(p):
                    # both heads' q/k transposes land in one PSUM tile so a
                    # single wide DVE copy moves them to SBUF (2x bf16 mode)
                    tp6 = ptp.tile([C, 6 * D], BF16, tag="tp6")
                    for i, h in enumerate((p, p + 1)):
                        hs = bass.ts(h, D)
                        nc.tensor.transpose(tp6[:, (2 * i) * D:(2 * i + 1) * D],
                                            phiq[:, hs], id_s[:])
                        nc.tensor.transpose(tp6[:, (2 * i + 1) * D:(2 * i + 2) * D],
                                            phik[:, hs], id_s[:])
                    qkT = sml.tile([C, 4 * D], BF16, tag="qkT")
                    nc.vector.tensor_copy(qkT[:], tp6[:, 0:4 * D])
                    return tp6, qkT

                def head_A(h, i, st):
                    tp6, qkT = st
                    nuA = pnu.tile([C, 3 * (D + 1) - 1], F32, tag="nuA")
                    Ar = nuA[:, 2 * (D + 1):3 * (D + 1) - 1]
                    nc.tensor.matmul(Ar[:], qkT[:, (2 * i + 1) * D:(2 * i + 2) * D],
                                     qkT[:, (2 * i) * D:(2 * i + 1) * D],
                                     start=True, stop=True)
                    A_sb = sml.tile([C, C], BF16, tag="A")
                    nc.vector.scalar_tensor_tensor(
                        out=A_sb[:], in0=Ar[:],
                        scalar=ainv_sb[:, h:h + 1], in1=mask_t[:],
                        op0=Alu.mult, op1=Alu.mult)
                    return (*st, nuA, A_sb)

                def head_nu(h, i, st):
                    tp6, qkT, nuA, A_sb = st
                    nu = nuA[:, 0:D + 1]
                    U = nuA[:, D + 1:2 * (D + 1)]
                    vh = v_sb[:, h * (D + 1):(h + 1) * (D + 1)]
                    # the A_sb-consuming matmul opens the bank's accumulation
                    # group: it naturally orders after the DVE mask, so the
                    # Ar readback completes before the bank is re-zeroed
                    nc.tensor.matmul(nu[:], A_sb[:], vh, start=True, stop=False)
                    nc.tensor.matmul(U[:], khats[h], vh, start=False,
                                     stop=False)
                    nc.tensor.matmul(nu[:], qkT[:, (2 * i) * D:(2 * i + 1) * D],
                                     S_cur[h][:], start=False, stop=True)
                    # recip/ybf first: the y-transpose unblocks before the
                    # (slack-tolerant) S update; denom = phi_q . z is strictly
                    # positive (phi > 0) so the reference's +eps (~1e-8
                    # relative) is dropped and the reciprocal reads PSUM
                    rd = sml.tile([C, 1], F32, tag="rd")
                    nc.vector.reciprocal(rd[:], nu[:, D:D + 1])
                    y_bf = sml.tile([C, D], BF16, tag="ybf")
                    nc.scalar.activation(y_bf[:], nu[:, 0:D], Act.Copy,
                                         scale=rd[:])
                    S_new = spool.tile([C, D + 1], BF16, tag=f"s{h}")
                    nc.vector.scalar_tensor_tensor(
                        out=S_new[:], in0=S_cur[h][:], scalar=acb_sb[:, h:h + 1],
                        in1=U, op0=Alu.mult, op1=Alu.add)
                    S_cur[h] = S_new
                    return (*st, y_bf)

                def head_yT(h, i, st):
                    tp6, y_bf = st[0], st[-1]
                    nc.tensor.transpose(tp6[:, (4 + i) * D:(5 + i) * D],
                                        y_bf[:], id_s[:])

                # out-projection of the PREVIOUS chunk is interleaved into the
                # scan as PE filler work behind the DVE/ACT dependency chains
                for p in (0, 2):
                    st = pair_tp(p)
                    s0 = head_A(p, 0, st)
                    s1 = head_A(p + 1, 1, st)
                    s0 = head_nu(p, 0, s0)
                    s1 = head_nu(p + 1, 1, s1)
                    if po_prev is not None:
                        emit_po(po_prev[0], po_prev[1],
                                orange=(0, 1, 2) if p == 0 else (3,))
                    head_yT(p, 0, s0)
                    head_yT(p + 1, 1, s1)
                    tp6 = st[0]
                    ys = slice(p * C, (p + 2) * C)
                    nc.scalar.copy(yt8[:, ys], tp6[:, 4 * D:6 * D])
                    if use_ytl:
                        nc.vector.tensor_tensor(out=ytl[:, ys],
                                                in0=tp6[:, 4 * D:6 * D],
                                                in1=yt8[:, ys], op=Alu.subtract)
                po_prev = ((yt8, ytl), c)

            # ---- chunk pipeline: projections run two chunks ahead of the
            # scan so their elementwise chains overlap earlier scans ----
            pend = [proj_phase(0)]
            if nch > 1:
                pend.append(proj_phase(1))
            for c in range(nch):
                scan_phase(c, pend.pop(0), last=(c == nch - 1))
                if c + 2 < nch:
                    pend.append(proj_phase(c + 2))

            emit_po(po_prev[0], po_prev[1], final=True)

    nc.compile()
    return nc


def _get_nc(cfg_key="default", **cfg):
    if cfg_key not in _CACHE:
        _CACHE[cfg_key] = _build(cfg)
    return _CACHE[cfg_key]


def _blk(m, fw):
    # [HID, fw] -> [C, NK*fw] with block k = m[k*128:(k+1)*128, :]
    return np.ascontiguousarray(
        m.reshape(NK, C, fw).transpose(1, 0, 2).reshape(C, NK * fw))


def _w3(W, fw, nterm):
    """W [HID, fw] f32 -> [C, nterm*NK*fw] e4m3: fp8(32W) | fp8(2W) | fp8(32Wl)."""
    W = np.asarray(W, np.float32)
    t0 = (SCALE * W).astype(E4)
    terms = [_blk(t0, fw), _blk((2.0 * W).astype(E4), fw)]
    if nterm == 3:
        wl = W - t0.astype(np.float32) / SCALE
        terms.append(_blk((SCALE * wl).astype(E4), fw))
    return np.ascontiguousarray(np.concatenate(terms[:nterm], axis=1))


def _wo3(Wo):
    """Wo [GW, HID] f32 -> [C, 2*HG*HID] e4m3, blocked by head: fp8(32Wo)
    (shared by the y8 and y-residual terms, both at 32x scale) | fp8(32*Wol)."""
    Wo = np.asarray(Wo, np.float32)
    t0 = (SCALE * Wo).astype(E4)
    wl = Wo - t0.astype(np.float32) / SCALE
    t2 = (SCALE * wl).astype(E4)

    def blk(m):
        return m.reshape(HG, C, HID).transpose(1, 0, 2).reshape(C, HG * HID)

    return np.ascontiguousarray(np.concatenate([blk(t0), blk(t2)], axis=1))


def make_in_maps(x, Wq, Wk, Wv, Wg, bg, Wo, bo):
    cosr, sinr = _rope_tables()
    maskT = np.triu(np.ones((C, C), np.float32))
    ident = np.eye(C, dtype=np.float32)
    x = np.asarray(x, np.float32)
    Wq, Wk, Wv = np.asarray(Wq), np.asarray(Wk), np.asarray(Wv)
    Wg, bg, Wo = np.asarray(Wg), np.asarray(bg), np.asarray(Wo)
    in_maps = []
    xTb_cache = {}
    for core in range(NCORES):
        b, hg = divmod(core, 4)
        cs = slice(hg * GW, (hg + 1) * GW)
        hsl = slice(hg * HG, (hg + 1) * HG)
        if b not in xTb_cache:
            # xTb[c, p, k*128+f] = x[b][c*128+f, k*128+p]; fp8 + 16*residual
            xT = np.ascontiguousarray(
                x[b].reshape(NCHUNK, C, NK, C).transpose(0, 3, 2, 1)
                .reshape(NCHUNK, C, HID)).astype(np.float32)
            x8 = xT.astype(E4)
            xl8 = (XL_S * (xT - x8.astype(np.float32))).astype(E4)
            xTb_cache[b] = np.ascontiguousarray(
                np.concatenate([x8, xl8], axis=2))
        in_maps.append({
            "xTb": xTb_cache[b],
            "wq3": _w3(Wq[:, cs], GW, WTERMS["q"]),
            "wk3": _w3(Wk[:, cs], GW, WTERMS["k"]),
            "wv3": _w3(Wv[:, cs], GW, WTERMS["v"]),
            "wg3": _w3(Wg[:, hsl], HG, WTERMS["g"]),
            "wo3": _wo3(Wo[cs, :]),
            "nbg4": np.tile(np.exp(-bg[None, hsl]), (C, 1)).astype(np.float32),
            "cosr": cosr, "sinr": sinr,
            "maskT": maskT, "ident": ident,
        })
    return in_maps


def kernel(x, Wq, Wk, Wv, Wg, bg, Wo, bo, _trace=False, **cfg):
    from concourse.bass_utils import run_bass_kernel_spmd
    nc = _get_nc(**cfg)
    in_maps = make_in_maps(x, Wq, Wk, Wv, Wg, bg, Wo, bo)
    res = run_bass_kernel_spmd(nc, in_maps, core_ids=list(range(NCORES)),
                               trace=_trace)
    out = np.zeros((B, L, HID), np.float32)
    for core in range(NCORES):
        b = core // 4
        out[b] += res.results[core]["out"].astype(np.float32)
    out += np.asarray(bo, np.float32)[None, None, :]
    kernel._last_results = res
    return out



# revision 61
# speedup vs baseline: 1.0003x; 1.0003x over previous
"""Trainium2 Bass kernel for nn_DeltaNet_22488448762128 (v3).

Full-input contract: kernel(**inputs) takes the unsharded numpy inputs and
returns the full [B, L, HID] output. Internally shards across 8 NeuronCores:
core = (b, hg) with b in {0,1} and hg in {0..3} head-groups of 4 heads.
Each core computes projections for its 4 heads, a chunked (C=128) linear
attention scan, and a partial output projection; the host sums the 4 partial
outputs per batch element and adds bo.

Math (per head, chunk c of size C, state S aug with z column):
  a_t   = cumprod(beta) within chunk;  aC = a_{C-1}
  q~_t  = phi(rope(q))_t * a_t ;  k^_s = phi(rope(k))_s * aC / a_s
  A^T[s,t] = (phi_k_s . q~_t) * (1/a_s) * [s<=t]
  nu    = A^T.T @ [V|1] + q~ @ S_aug   ;  y_t = nu[:, :D] / (nu[:, D] + eps)
  S_aug = aC * S_aug + k^T @ [V|1]

Precision: projections and the output projection run on the PE in fp8 (e4m3)
DoubleRow mode (pairs two 128-row contraction tiles per pass) with
error-compensated term stacks
  x @ W  =  x8 @ W8  [+ xl8 @ W8']  [+ x8 @ Wl8]
(x8 = fp8(x), xl8 = fp8(16*(x - x8)), Wl = W - fp8(32W)/32). Measured
per-path sensitivity sets the term counts (WTERMS): q/k run 1-term (their
quantization noise cancels through the normalized attention), v and the
output projection need all terms (their noise reaches y linearly), g is
free. Weights are pre-scaled by 32 into e4m3's normal range; the inverse
scale is folded into the rope tables, the sigmoid's activation scale, the
augmented-ones column (=32), and the final output-copy scale, so unscaling
costs zero extra instructions. The reference's +eps (1e-6) on the strictly
positive denominator is dropped. The scan runs in bf16 operands with fp32
PSUM accumulation; the rope/phi elementwise path is bf16 end-to-end (2-4x
DVE modes). Output is returned bf16; the host sums partials in fp32.

PSUM banks (8): pj[big x2] g/q/k/v projection groups (+ final-po overflow);
ppo[po x1] output projection; ptp[tp6 x2] per-head-PAIR q/k/y transposes
(one wide 2x-mode DVE copy per pair instead of per-head ACT copies);
pnu[nuA x2] per-head A|nu|U groups (A's group is closed before the nu/U
group reopens the bank; the reopening matmul consumes A_sb, so it naturally
orders after the DVE mask's Ar readback); pbeta[bt x1] the beta-chain
transposes, decoupled from the scan's pnu ring so the two-chunk-ahead
projection phase can't stall the scan.

Schedule: two-level software pipeline. Outer: proj_phase(c+2) (projections,
beta chain, rope/phi, khat) is emitted two chunks ahead of scan_phase(c),
so the long rope->phi elementwise chains overlap earlier chunks' scans and
the PE never waits on them in steady state. Inner: 2-wide pipelined head
pairs; the previous chunk's output projection is interleaved as PE filler
(o-tiles 0-2 at the first pair, 3 at the second); the final chunk's po
alternates ppo/pj banks so its groups overlap their readbacks. The rope
tables are fetched once (not per chunk); small constants are deferred past
wq on the serialized HWDGE so chunk 0's projections start ~1.5us earlier.
GPSIMD (no PSUM access, no stt opcode) carries SBUF-local q-side work:
rope(q)'s second rotation half, phi(q)'s min and its relu+add assembly.
The y scale (1/denom) is applied on ScalarE (activation Copy with a
per-partition scale pointer), freeing DVE for the PSUM-bound scan ops.
"""

import math
import numpy as np
import ml_dtypes

B, L, HID = 2, 2048, 2048
H, D = 16, 128
HG = 4              # heads per core
C = 128             # chunk size
NCHUNK = L // C     # 16
NK = HID // C       # 16 contraction tiles
NKP = NK // 2       # 8 DoubleRow pair-tiles
EPS = 1e-6
BETA_MIN, BETA_MAX = 0.8, 0.9995
NCORES = 8
GW = HG * D         # 512, per-core projection width
NO = HID // GW      # 4 output col tiles
SCALE = 32.0        # fp8 weight pre-scale
XL_S = 16.0         # x residual pre-scale
E4 = ml_dtypes.float8_e4m3

# fp8 GEMM term counts: 2 = x8@W8 + xl8@W8' (x-quantization compensated),
# 3 = + x8@Wl8 (weight-quantization compensated too). g is ~free (N=4) so it
# keeps 3 terms; q/k/v tolerate W-quant noise (it largely cancels or averages
# out through the normalized attention).
WTERMS = {"q": 1, "k": 1, "v": 3, "g": 3}
PO_SLOTS = ((0, 1), (0, 0), (1, 0))  # (wo term slot, y source: 0=y8 1=ylow)

_CACHE = {}


def _rope_tables():
    half = D // 2
    inv_freq = (1.0 / (10000.0 ** (np.arange(half, dtype=np.float32) /
                                   np.float32(half)))).astype(np.float32)
    t = np.arange(L, dtype=np.float32)
    freqs = t[:, None] * inv_freq[None, :]
    # fold the fp8 weight pre-scale out of q/k here: tables are cos/32, sin/32
    cos = (np.cos(freqs) / SCALE).astype(ml_dtypes.bfloat16)   # [L, 64]
    sin = (np.sin(freqs) / SCALE).astype(ml_dtypes.bfloat16)
    # chunk-major: [128, NCHUNK*64], block c = rows c*128..c*128+128
    def rearr(m):
        return np.ascontiguousarray(
            m.reshape(NCHUNK, C, half).transpose(1, 0, 2).reshape(C, NCHUNK * half))
    return rearr(cos), rearr(sin)


def _build(cfg):
    import concourse.bass as bass
    import concourse.bacc as bacc
    import concourse.tile as tile
    import concourse.mybir as mybir
    from contextlib import ExitStack

    dt = mybir.dt
    F32 = dt.float32
    BF16 = dt.bfloat16
    F8 = dt.float8e4
    DRm = mybir.MatmulPerfMode.DoubleRow
    Alu = mybir.AluOpType
    Act = mybir.ActivationFunctionType
    half = D // 2

    nch = cfg.get("nchunk", NCHUNK)

    nc = bacc.Bacc("TRN2", target_bir_lowering=False, debug=False,
                   enable_asserts=False, num_devices=NCORES)

    # ---- DRAM I/O (host passes PE-blocked layouts, see make_in_maps) ----
    xT_d = nc.dram_tensor("xTb", [NCHUNK, C, 2 * HID], F8, kind="ExternalInput").ap()
    wq_d = nc.dram_tensor("wq3", [C, WTERMS["q"] * NK * GW], F8,
                          kind="ExternalInput").ap()
    wk_d = nc.dram_tensor("wk3", [C, WTERMS["k"] * NK * GW], F8,
                          kind="ExternalInput").ap()
    wv_d = nc.dram_tensor("wv3", [C, WTERMS["v"] * NK * GW], F8,
                          kind="ExternalInput").ap()
    wg_d = nc.dram_tensor("wg3", [C, WTERMS["g"] * NK * HG], F8,
                          kind="ExternalInput").ap()
    wo_d = nc.dram_tensor("wo3", [C, 2 * HG * HID], F8, kind="ExternalInput").ap()
    nbg_d = nc.dram_tensor("nbg4", [C, HG], F32, kind="ExternalInput").ap()
    cos_d = nc.dram_tensor("cosr", [C, NCHUNK * half], BF16, kind="ExternalInput").ap()
    sin_d = nc.dram_tensor("sinr", [C, NCHUNK * half], BF16, kind="ExternalInput").ap()
    mask_d = nc.dram_tensor("maskT", [C, C], F32, kind="ExternalInput").ap()
    id_d = nc.dram_tensor("ident", [C, C], F32, kind="ExternalInput").ap()
    out_d = nc.dram_tensor("out", [L, HID], BF16, kind="ExternalOutput").ap()

    def pair(t, off, step, f):
        b = t[:]
        return bass.AP(tensor=b.tensor, offset=b.offset + off,
                       ap=[b.ap[0], [step, 2], [1, f]])

    with ExitStack() as ctx:
        tc = ctx.enter_context(tile.TileContext(nc))

        cpool = ctx.enter_context(tc.tile_pool(name="consts", bufs=1))
        mask_t = cpool.tile([C, C], F32, tag="mask")
        id_t = cpool.tile([C, C], F32, tag="id")
        id_s = cpool.tile([C, C], BF16, tag="id_s")
        ones_t = cpool.tile([C, C], F32, tag="ones")
        nbg_t = cpool.tile([C, HG], F32, tag="nbg")
        nc.vector.memset(ones_t[:], 1.0)

        with ExitStack() as main:
            wpool = main.enter_context(tc.tile_pool(name="w", bufs=1))
            wq_t = wpool.tile([C, WTERMS["q"] * NK * GW], F8, tag="wq")
            wk_t = wpool.tile([C, WTERMS["k"] * NK * GW], F8, tag="wk")
            wv_t = wpool.tile([C, WTERMS["v"] * NK * GW], F8, tag="wv")
            wg_t = wpool.tile([C, WTERMS["g"] * NK * HG], F8, tag="wg")
            wo_t = wpool.tile([C, 2 * HG * HID], F8, tag="wo")
            nc.sync.dma_start(wg_t[:], wg_d)

            # chunk-local SBUF pools
            xp = main.enter_context(tc.tile_pool(name="xp", bufs=cfg.get("xp", 3)))
            cspool = main.enter_context(tc.tile_pool(name="cspool", bufs=cfg.get("csp", 6)))

            def fetch_x(t, ci, eng=None):
                eng = eng or nc.scalar
                eng.dma_start(t[:, 0:HID], xT_d[ci][:, 0:HID])
                eng.dma_start(t[:, HID:2 * HID], xT_d[ci][:, HID:2 * HID])

            # whole-kernel rope tables fetched once (2 DMAs instead of 32):
            # per-chunk cos/sin are views into these resident tiles
            costab = cspool.tile([C, NCHUNK * half], BF16, tag="costab", bufs=1)
            sintab = cspool.tile([C, NCHUNK * half], BF16, tag="sintab", bufs=1)

            def fetch_cs(ci, eng=None):
                return (costab[:, bass.ts(ci, half)],
                        sintab[:, bass.ts(ci, half)])

            # prefetch chunk 0/1 x + rope tables ahead of the weight
            # stream so chunk 0 isn't queued behind 9 MB of weights
            xpre = []
            cspre = []
            for cpre in range(min(2, nch)):
                t = xp.tile([C, 2 * HID], F8, tag="xtb")
                xpre.append(t)
            fetch_x(xpre[0], 0)
            cspre.append(fetch_cs(0))

            # weights streamed in PE consumption order (g,q,k,v then wo),
            # sliced so the PE can trail the DMA k-pair by k-pair; small
            # constants deferred past wq so chunk 0's q projection isn't
            # queued behind them on the serialized HWDGE
            TW = NK * GW

            def wslices(w_t, w_d, nt):
                for term in range(nt):
                    for hf in range(2):
                        sl = slice(term * TW + hf * TW // 2,
                                   term * TW + (hf + 1) * TW // 2)
                        nc.sync.dma_start(w_t[:, sl], w_d[:, sl])

            wslices(wq_t, wq_d, WTERMS["q"])
            nc.scalar.dma_start(costab[:], cos_d)
            nc.scalar.dma_start(sintab[:], sin_d)
            nc.sync.dma_start(id_t[:], id_d)
            nc.sync.dma_start(nbg_t[:], nbg_d)
            nc.scalar.copy(id_s[:], id_t[:])
            cspre.append(fetch_cs(1))
            wslices(wk_t, wk_d, WTERMS["k"])
            fetch_x(xpre[1], 1)
            nc.sync.dma_start(mask_t[:], mask_d)
            wslices(wv_t, wv_d, WTERMS["v"])
            for term in (0, 1):
                ts_ = bass.ts(term, HG * HID)
                nc.sync.dma_start(wo_t[:, ts_], wo_d[:, ts_])
            big2 = main.enter_context(tc.tile_pool(name="big2", bufs=cfg.get("big2", 2)))
            sml = main.enter_context(tc.tile_pool(name="sml", bufs=cfg.get("sml", 4)))
            spool = main.enter_context(tc.tile_pool(name="spool", bufs=2))
            ypool = main.enter_context(tc.tile_pool(name="ypool", bufs=2))
            osb = main.enter_context(tc.tile_pool(name="osb", bufs=cfg.get("osb", 5)))

            # psum pools: pj 2 + ppo 2 + ptp 2 + pnu 2 = 8 banks
            pj = main.enter_context(tc.tile_pool(
                name="pj", bufs=cfg.get("pj", 2), space="PSUM"))
            ppo = main.enter_context(tc.tile_pool(
                name="ppo", bufs=cfg.get("ppo", 1), space="PSUM"))
            ptp = main.enter_context(tc.tile_pool(
                name="ptp", bufs=cfg.get("ptp", 2), space="PSUM"))
            pnu = main.enter_context(tc.tile_pool(
                name="pnu", bufs=cfg.get("pnu", 2), space="PSUM"))
            pbeta = main.enter_context(tc.tile_pool(
                name="pbeta", bufs=cfg.get("pbeta", 1), space="PSUM"))

            S_cur = []
            for h in range(HG):
                s0 = spool.tile([C, D + 1], BF16, tag=f"s{h}")
                nc.vector.memset(s0[:], 0.0)
                S_cur.append(s0)

            def proj_mms(ps, fw, w_t, tw, nterm, xtb):
                n = 0
                for term in range(nterm):
                    xoff = HID if term == 1 else 0
                    for kp in range(NKP):
                        nc.tensor.matmul(
                            ps[:, 0:fw],
                            pair(xtb, xoff + kp * 2 * C, C, C),
                            pair(w_t, term * tw + kp * 2 * fw, fw, fw),
                            start=(n == 0), stop=(n == nterm * NKP - 1),
                            perf_mode=DRm)
                        n += 1

            def rope(src, dst, tmp, cs, tmp2=None):
                # tmp2 set: de-half on DVE, do-half on GPSIMD concurrently
                cos_c, sin_c = cs
                ed = nc.vector
                eo = nc.gpsimd if tmp2 is not None else nc.vector
                se = src[:].rearrange("p (h d) -> p h d", h=HG)[:, :, 0:half]
                so = src[:].rearrange("p (h d) -> p h d", h=HG)[:, :, half:D]
                de = dst[:].rearrange("p (h d) -> p h d", h=HG)[:, :, 0:half]
                do = dst[:].rearrange("p (h d) -> p h d", h=HG)[:, :, half:D]
                cc = bass.AP(tensor=cos_c.tensor, offset=cos_c.offset,
                             ap=[cos_c.ap[0], [0, HG], [1, half]])
                ss = bass.AP(tensor=sin_c.tensor, offset=sin_c.offset,
                             ap=[sin_c.ap[0], [0, HG], [1, half]])
                t1 = tmp[:].rearrange("p (h d) -> p h d", h=HG)[:, :, 0:half]
                t2 = tmp[:].rearrange("p (h d) -> p h d", h=HG)[:, :, half:D]
                tb = tmp2 if tmp2 is not None else tmp
                t3 = tb[:].rearrange("p (h d) -> p h d", h=HG)[:, :, 0:half]
                t4 = tb[:].rearrange("p (h d) -> p h d", h=HG)[:, :, half:D]
                ed.tensor_tensor(out=t1, in0=se, in1=cc, op=Alu.mult)
                ed.tensor_tensor(out=t2, in0=so, in1=ss, op=Alu.mult)
                ed.tensor_tensor(out=de, in0=t1, in1=t2, op=Alu.subtract)
                eo.tensor_tensor(out=t3, in0=se, in1=ss, op=Alu.mult)
                eo.tensor_tensor(out=t4, in0=so, in1=cc, op=Alu.mult)
                eo.tensor_tensor(out=do, in0=t3, in1=t4, op=Alu.add)

            po_prev = None  # (yt8, ytl) of previous chunk

            def emit_po(ysrcs, c_out, orange=None, final=False):
                yt8_, ytl_ = ysrcs
                nmm = 2 * len(PO_SLOTS)
                for o in (orange if orange is not None else range(NO)):
                    # in the epilogue the projection banks are idle: alternate
                    # pools so o-tile groups overlap their readback copies
                    if final and o % 2 == 1:
                        out_ps = pj.tile([C, GW], F32, tag="big")
                    else:
                        out_ps = ppo.tile([C, GW], F32, tag="po")
                    n = 0
                    # hp-outer: the first MMs only need heads 0-1, so the
                    # group can start before heads 2-3 finish
                    for hp in range(HG // 2):
                        for slot, ysel in PO_SLOTS:
                            ysrc = yt8_ if ysel == 0 else ytl_
                            nc.tensor.matmul(
                                out_ps[:],
                                pair(ysrc, hp * 2 * C, C, C),
                                pair(wo_t,
                                     slot * HG * HID + (2 * hp) * HID + o * GW,
                                     HID, GW),
                                start=(n == 0), stop=(n == nmm - 1),
                                perf_mode=DRm)
                            n += 1
                    out_sb = osb.tile([C, GW], BF16, tag="osb")
                    if o % 2 == 0:
                        nc.scalar.mul(out_sb[:], out_ps[:], 1.0 / SCALE)
                    else:
                        nc.vector.tensor_scalar_mul(out_sb[:], out_ps[:],
                                                    1.0 / SCALE)
                    nc.sync.dma_start(out_d[bass.ts(c_out, C), bass.ts(o, GW)],
                                      out_sb[:])

            def proj_phase(c):
                """Projections + beta chain + rope/phi/khat for chunk c.
                Emitted two chunks ahead of the scan so the elementwise
                chains overlap earlier chunks' scans."""
                if c < len(xpre):
                    xtb = xpre[c]
                    cos_c, sin_c = cspre[c]
                else:
                    xtb = xp.tile([C, 2 * HID], F8, tag="xtb")
                    fetch_x(xtb, c)
                    cos_c, sin_c = fetch_cs(c)

                # ---- projections (PE) interleaved with beta chain ----
                # g first so the long beta dependency chain starts early
                g_ps = pj.tile([C, GW], F32, tag="big")
                proj_mms(g_ps, HG, wg_t, NK * HG, WTERMS["g"], xtb)
                beta_sb = sml.tile([C, HG], F32, tag="beta")
                nc.scalar.activation(beta_sb[:], g_ps[:, 0:HG], Act.Exp,
                                     scale=-1.0 / SCALE)
                nc.vector.scalar_tensor_tensor(
                    out=beta_sb[:], in0=beta_sb[:], scalar=1.0,
                    in1=nbg_t[:], op0=Alu.mult, op1=Alu.mult)
                nc.vector.tensor_scalar_add(beta_sb[:], beta_sb[:], 1.0)
                nc.vector.reciprocal(beta_sb[:], beta_sb[:])
                nc.vector.tensor_scalar(out=beta_sb[:], in0=beta_sb[:],
                                        scalar1=BETA_MIN, scalar2=BETA_MAX,
                                        op0=Alu.max, op1=Alu.min)

                q_ps = pj.tile([C, GW], F32, tag="big")
                proj_mms(q_ps, GW, wq_t, TW, WTERMS["q"], xtb)
                q_sb = big2.tile([C, GW], BF16, tag="q")
                nc.scalar.copy(q_sb[:], q_ps[:])

                # beta transposes ride the pnu rotation between head uses
                btp_ps = pbeta.tile([C, C], F32, tag="bt")
                nc.tensor.transpose(btp_ps[0:HG, 0:C], beta_sb[:], id_t[:])
                btp_sb = sml.tile([HG, C], F32, tag="btp")
                nc.scalar.copy(btp_sb[:], btp_ps[0:HG, 0:C])
                aT_sb = sml.tile([HG, C], F32, tag="aT")
                nc.vector.tensor_tensor_scan(
                    out=aT_sb[:], data0=btp_sb[:], data1=ones_t[0:HG, :],
                    initial=1.0, op0=Alu.mult, op1=Alu.mult)

                k_ps = pj.tile([C, GW], F32, tag="big")
                proj_mms(k_ps, GW, wk_t, TW, WTERMS["k"], xtb)
                k_sb = big2.tile([C, GW], BF16, tag="k")
                nc.scalar.copy(k_sb[:], k_ps[:])

                a_ps = pbeta.tile([C, C], F32, tag="bt")
                nc.tensor.transpose(a_ps[:, 0:HG], aT_sb[:], id_t[0:HG, 0:HG])
                a_sb = sml.tile([C, HG], F32, tag="a")
                nc.scalar.copy(a_sb[:], a_ps[:, 0:HG])
                ainv_sb = sml.tile([C, HG], F32, tag="ainv")
                nc.vector.reciprocal(ainv_sb[:], a_sb[:])
                diag4 = sml.tile([HG, HG], F32, tag="diag4")
                nc.vector.tensor_scalar(out=diag4[:], in0=id_t[0:HG, 0:HG],
                                        scalar1=aT_sb[:, C - 1:C], scalar2=None,
                                        op0=Alu.mult)

                # rope(q) early on DVE; phi(q) min + assembly on GPSIMD
                # (the q path is off the serial S/z recurrence, so the slow
                # Pool engine can carry it)
                qr = big2.tile([C, GW], BF16, tag="qr")
                rtq = big2.tile([C, GW], BF16, tag="rtq")
                rtq2 = big2.tile([C, GW], BF16, tag="rtq2")
                rope(q_sb, qr, rtq, (cos_c, sin_c), rtq2)
                tmq = big2.tile([C, GW], BF16, tag="mq")
                nc.gpsimd.tensor_scalar_min(tmq[:], qr[:], 0.0)
                teq = big2.tile([C, GW], BF16, tag="eq")
                nc.scalar.activation(teq[:], tmq[:], Act.Exp)
                # phi(q) assembled on GPSIMD (no stt opcode there: relu + add)
                rlq = big2.tile([C, GW], BF16, tag="rlq")
                nc.gpsimd.tensor_scalar_max(rlq[:], qr[:], 0.0)
                phiq = big2.tile([C, GW], BF16, tag="phq", bufs=4)
                nc.gpsimd.tensor_tensor(out=phiq[:], in0=rlq[:], in1=teq[:],
                                        op=Alu.add)

                v_ps = pj.tile([C, GW], F32, tag="big")
                proj_mms(v_ps, GW, wv_t, TW, WTERMS["v"], xtb)
                v_sb = big2.tile([C, HG * (D + 1)], BF16, tag="v", bufs=4)
                v_aug = v_sb[:].rearrange("p (h e) -> p h e", e=D + 1)
                nc.scalar.copy(v_aug[:, :, 0:D],
                               v_ps[:, 0:GW].rearrange("p (h e) -> p h e", e=D))
                nc.vector.memset(v_aug[:, :, D:D + 1], SCALE)

                acb_ps = pbeta.tile([C, C], F32, tag="bt")
                nc.tensor.matmul(acb_ps[:, 0:HG], ones_t[0:HG, :], diag4[:],
                                 start=True, stop=True)
                acb_sb = sml.tile([C, HG], F32, tag="acb")
                nc.scalar.copy(acb_sb[:], acb_ps[:, 0:HG])
                acdiv_sb = sml.tile([C, HG], F32, tag="acdiv")
                nc.vector.tensor_tensor(out=acdiv_sb[:], in0=ainv_sb[:],
                                        in1=acb_sb[:], op=Alu.mult)

                # rope(k) + phi(k) stay on the fast engines: the k-path feeds
                # the serial S/z recurrence (khat -> U -> S_new)
                kr = big2.tile([C, GW], BF16, tag="kr")
                rtk = big2.tile([C, GW], BF16, tag="rtk")
                rope(k_sb, kr, rtk, (cos_c, sin_c))
                tmk = big2.tile([C, GW], BF16, tag="mk")
                nc.vector.tensor_scalar_min(tmk[:], kr[:], 0.0)
                tek = big2.tile([C, GW], BF16, tag="ek")
                nc.scalar.activation(tek[:], tmk[:], Act.Exp)
                phik = big2.tile([C, GW], BF16, tag="phk", bufs=4)
                nc.vector.scalar_tensor_tensor(out=phik[:], in0=kr[:],
                                               scalar=0.0, in1=tek[:],
                                               op0=Alu.max, op1=Alu.add)

                # khat for all heads, hoisted off the per-head critical path
                khats = []
                for h in range(HG):
                    khat = sml.tile([C, D], BF16, tag="khat", bufs=12)
                    nc.vector.tensor_scalar_mul(khat[:], phik[:, bass.ts(h, D)],
                                                acdiv_sb[:, h:h + 1])
                    khats.append(khat[:])

                return dict(phiq=phiq, phik=phik, khats=khats, v_sb=v_sb,
                            ainv_sb=ainv_sb, acb_sb=acb_sb)

            def scan_phase(c, P, last=False):
                nonlocal po_prev
                phiq, phik = P["phiq"], P["phik"]
                khats, v_sb = P["khats"], P["v_sb"]
                ainv_sb, acb_sb = P["ainv_sb"], P["acb_sb"]

                # ---- scan, 2-wide head pipeline ----
                use_ytl = any(ysel == 1 for _, ysel in PO_SLOTS)
                yt8 = ypool.tile([C, HG * C], F8, tag="yt8")
                if use_ytl:
                    ytl = ypool.tile([C, HG * C], F8, tag="ytl")
                else:
                    ytl = None

                def pair_tp(p):
                    # both heads' q/k transposes land in one PSUM tile so a
                    # single wide DVE copy moves them to SBUF (2x bf16 mode)
                    tp6 = ptp.tile([C, 6 * D], BF16, tag="tp6")
                    for i, h in enumerate((p, p + 1)):
                        hs = bass.ts(h, D)
                        nc.tensor.transpose(tp6[:, (2 * i) * D:(2 * i + 1) * D],
                                            phiq[:, hs], id_s[:])
                        nc.tensor.transpose(tp6[:, (2 * i + 1) * D:(2 * i + 2) * D],
                                            phik[:, hs], id_s[:])
                    qkT = sml.tile([C, 4 * D], BF16, tag="qkT")
                    nc.vector.tensor_copy(qkT[:], tp6[:, 0:4 * D])
                    return tp6, qkT

                def head_A(h, i, st):
                    tp6, qkT = st
                    nuA = pnu.tile([C, 3 * (D + 1) - 1], F32, tag="nuA")
                    Ar = nuA[:, 2 * (D + 1):3 * (D + 1) - 1]
                    nc.tensor.matmul(Ar[:], qkT[:, (2 * i + 1) * D:(2 * i + 2) * D],
                                     qkT[:, (2 * i) * D:(2 * i + 1) * D],
                                     start=True, stop=True)
                    A_sb = sml.tile([C, C], BF16, tag="A")
                    nc.vector.scalar_tensor_tensor(
                        out=A_sb[:], in0=Ar[:],
                        scalar=ainv_sb[:, h:h + 1], in1=mask_t[:],
                        op0=Alu.mult, op1=Alu.mult)
                    return (*st, nuA, A_sb)

                def head_nu(h, i, st):
                    tp6, qkT, nuA, A_sb = st
                    nu = nuA[:, 0:D + 1]
                    U = nuA[:, D + 1:2 * (D + 1)]
                    vh = v_sb[:, h * (D + 1):(h + 1) * (D + 1)]
                    # the A_sb-consuming matmul opens the bank's accumulation
                    # group: it naturally orders after the DVE mask, so the
                    # Ar readback completes before the bank is re-zeroed
                    nc.tensor.matmul(nu[:], A_sb[:], vh, start=True, stop=False)
                    nc.tensor.matmul(U[:], khats[h], vh, start=False,
                                     stop=False)
                    nc.tensor.matmul(nu[:], qkT[:, (2 * i) * D:(2 * i + 1) * D],
                                     S_cur[h][:], start=False, stop=True)
                    # recip/ybf first: the y-transpose unblocks before the
                    # (slack-tolerant) S update; denom = phi_q . z is strictly
                    # positive (phi > 0) so the reference's +eps (~1e-8
                    # relative) is dropped and the reciprocal reads PSUM
                    rd = sml.tile([C, 1], F32, tag="rd")
                    nc.vector.reciprocal(rd[:], nu[:, D:D + 1])
                    y_bf = sml.tile([C, D], BF16, tag="ybf")
                    nc.scalar.activation(y_bf[:], nu[:, 0:D], Act.Copy,
                                         scale=rd[:])
                    S_new = spool.tile([C, D + 1], BF16, tag=f"s{h}")
                    nc.vector.scalar_tensor_tensor(
                        out=S_new[:], in0=S_cur[h][:], scalar=acb_sb[:, h:h + 1],
                        in1=U, op0=Alu.mult, op1=Alu.add)
                    S_cur[h] = S_new
                    return (*st, y_bf)

                def head_yT(h, i, st):
                    tp6, y_bf = st[0], st[-1]
                    nc.tensor.transpose(tp6[:, (4 + i) * D:(5 + i) * D],
                                        y_bf[:], id_s[:])

                # out-projection of the PREVIOUS chunk is interleaved into the
                # scan as PE filler work behind the DVE/ACT dependency chains
                for p in (0, 2):
                    st = pair_tp(p)
                    s0 = head_A(p, 0, st)
                    s1 = head_A(p + 1, 1, st)
                    s0 = head_nu(p, 0, s0)
                    s1 = head_nu(p + 1, 1, s1)
                    if po_prev is not None:
                        emit_po(po_prev[0], po_prev[1],
                                orange=(0, 1, 2) if p == 0 else (3,))
                    head_yT(p, 0, s0)
                    head_yT(p + 1, 1, s1)
                    tp6 = st[0]
                    ys = slice(p * C, (p + 2) * C)
                    nc.scalar.copy(yt8[:, ys], tp6[:, 4 * D:6 * D])
                    if use_ytl:
                        nc.vector.tensor_tensor(out=ytl[:, ys],
                                                in0=tp6[:, 4 * D:6 * D],
                                                in1=yt8[:, ys], op=Alu.subtract)
                po_prev = ((yt8, ytl), c)

            # ---- chunk pipeline: projections run two chunks ahead of the
            # scan so their elementwise chains overlap earlier scans ----
            pend = [proj_phase(0)]
            if nch > 1:
                pend.append(proj_phase(1))
            for c in range(nch):
                scan_phase(c, pend.pop(0), last=(c == nch - 1))
                if c + 2 < nch:
                    pend.append(proj_phase(c + 2))

            emit_po(po_prev[0], po_prev[1], final=True)

    nc.compile()
    return nc


def _get_nc(cfg_key="default", **cfg):
    if cfg_key not in _CACHE:
        _CACHE[cfg_key] = _build(cfg)
    return _CACHE[cfg_key]


def _blk(m, fw):
    # [HID, fw] -> [C, NK*fw] with block k = m[k*128:(k+1)*128, :]
    return np.ascontiguousarray(
        m.reshape(NK, C, fw).transpose(1, 0, 2).reshape(C, NK * fw))


def _w3(W, fw, nterm):
    """W [HID, fw] f32 -> [C, nterm*NK*fw] e4m3: fp8(32W) | fp8(2W) | fp8(32Wl)."""
    W = np.asarray(W, np.float32)
    t0 = (SCALE * W).astype(E4)
    terms = [_blk(t0, fw), _blk((2.0 * W).astype(E4), fw)]
    if nterm == 3:
        wl = W - t0.astype(np.float32) / SCALE
        terms.append(_blk((SCALE * wl).astype(E4), fw))
    return np.ascontiguousarray(np.concatenate(terms[:nterm], axis=1))


def _wo3(Wo):
    """Wo [GW, HID] f32 -> [C, 2*HG*HID] e4m3, blocked by head: fp8(32Wo)
    (shared by the y8 and y-residual terms, both at 32x scale) | fp8(32*Wol)."""
    Wo = np.asarray(Wo, np.float32)
    t0 = (SCALE * Wo).astype(E4)
    wl = Wo - t0.astype(np.float32) / SCALE
    t2 = (SCALE * wl).astype(E4)

    def blk(m):
        return m.reshape(HG, C, HID).transpose(1, 0, 2).reshape(C, HG * HID)

    return np.ascontiguousarray(np.concatenate([blk(t0), blk(t2)], axis=1))


def make_in_maps(x, Wq, Wk, Wv, Wg, bg, Wo, bo):
    cosr, sinr = _rope_tables()
    maskT = np.triu(np.ones((C, C), np.float32))
    ident = np.eye(C, dtype=np.float32)
    x = np.asarray(x, np.float32)
    Wq, Wk, Wv = np.asarray(Wq), np.asarray(Wk), np.asarray(Wv)
    Wg, bg, Wo = np.asarray(Wg), np.asarray(bg), np.asarray(Wo)
    in_maps = []
    xTb_cache = {}
    for core in range(NCORES):
        b, hg = divmod(core, 4)
        cs = slice(hg * GW, (hg + 1) * GW)
        hsl = slice(hg * HG, (hg + 1) * HG)
        if b not in xTb_cache:
            # xTb[c, p, k*128+f] = x[b][c*128+f, k*128+p]; fp8 + 16*residual
            xT = np.ascontiguousarray(
                x[b].reshape(NCHUNK, C, NK, C).transpose(0, 3, 2, 1)
                .reshape(NCHUNK, C, HID)).astype(np.float32)
            x8 = xT.astype(E4)
            xl8 = (XL_S * (xT - x8.astype(np.float32))).astype(E4)
            xTb_cache[b] = np.ascontiguousarray(
                np.concatenate([x8, xl8], axis=2))
        in_maps.append({
            "xTb": xTb_cache[b],
            "wq3": _w3(Wq[:, cs], GW, WTERMS["q"]),
            "wk3": _w3(Wk[:, cs], GW, WTERMS["k"]),
            "wv3": _w3(Wv[:, cs], GW, WTERMS["v"]),
            "wg3": _w3(Wg[:, hsl], HG, WTERMS["g"]),
            "wo3": _wo3(Wo[cs, :]),
            "nbg4": np.tile(np.exp(-bg[None, hsl]), (C, 1)).astype(np.float32),
            "cosr": cosr, "sinr": sinr,
            "maskT": maskT, "ident": ident,
        })
    return in_maps


def kernel(x, Wq, Wk, Wv, Wg, bg, Wo, bo, _trace=False, **cfg):
    from concourse.bass_utils import run_bass_kernel_spmd
    nc = _get_nc(**cfg)
    in_maps = make_in_maps(x, Wq, Wk, Wv, Wg, bg, Wo, bo)
    res = run_bass_kernel_spmd(nc, in_maps, core_ids=list(range(NCORES)),
                               trace=_trace)
    out = np.zeros((B, L, HID), np.float32)
    for core in range(NCORES):
        b = core // 4
        out[b] += res.results[core]["out"].astype(np.float32)
    out += np.asarray(bo, np.float32)[None, None, :]
    kernel._last_results = res
    return out



# revision 71
# speedup vs baseline: 1.0229x; 1.0226x over previous
"""Trainium2 Bass kernel for nn_DeltaNet_22488448762128 (v4).

Full-input contract: kernel(**inputs) takes the unsharded numpy inputs and
returns the full [B, L, HID] output. Internally shards across 8 NeuronCores:
core = (b, hg) with b in {0,1} and hg in {0..3} head-groups of 4 heads.
Each core computes projections for its 4 heads, a chunked (C=128) linear
attention scan, and a partial output projection; the host sums the 4 partial
outputs per batch element and adds bo.

Math (per head, chunk c of size C, state S aug with z column):
  a_t   = cumprod(beta) within chunk;  aC = a_{C-1}
  q~_t  = phi(rope(q))_t * a_t ;  k^_s = phi(rope(k))_s * aC / a_s
  A^T[s,t] = (phi_k_s . q~_t) * (1/a_s) * [s<=t]
  nu    = A^T.T @ [V|1] + q~ @ S_aug   ;  y_t = nu[:, :D] / (nu[:, D] + eps)
  S_aug = aC * S_aug + k^T @ [V|1]

Precision: projections and the output projection run on the PE in fp8 (e4m3)
DoubleRow mode (pairs two 128-row contraction tiles per pass) with
error-compensated term stacks
  x @ W  =  x8 @ W8  [+ xl8 @ W8']  [+ x8 @ Wl8]
(x8 = fp8(x), xl8 = fp8(16*(x - x8)), Wl = W - fp8(32W)/32). Measured
per-path sensitivity sets the term counts (WTERMS): q/k run 1-term (their
quantization noise cancels through the normalized attention), v and the
output projection need all terms (their noise reaches y linearly), g is
free. Weights are pre-scaled by 32 into e4m3's normal range; the inverse
scale is folded into the rope tables, the sigmoid's activation scale, the
augmented-ones column (=32), and the final output-copy scale, so unscaling
costs zero extra instructions. The reference's +eps (1e-6) on the strictly
positive denominator is dropped. The scan runs in bf16 operands with fp32
PSUM accumulation; the rope/phi elementwise path is bf16 end-to-end (2-4x
DVE modes). Output is returned bf16; the host sums partials in fp32.

PSUM banks (8): pj[big x2] g/q/k/v projection groups (+ final-po overflow);
ppo[po x1] output projection; ptp[tp6 x2] per-head-PAIR q/k/y transposes
(one wide 2x-mode DVE copy per pair instead of per-head ACT copies);
pnu[nuA x2] per-head A|nu|U groups (A's group is closed before the nu/U
group reopens the bank; the reopening matmul consumes A_sb, so it naturally
orders after the DVE mask's Ar readback); pbeta[bt x1] the beta-chain
transposes, decoupled from the scan's pnu ring so the two-chunk-ahead
projection phase can't stall the scan.

Schedule: two-level software pipeline. Outer: proj_phase(c+2) (projections,
beta chain, rope/phi, khat) is emitted two chunks ahead of scan_phase(c),
so the long rope->phi elementwise chains overlap earlier chunks' scans and
the PE never waits on them in steady state. Inner: 2-wide pipelined head
pairs; the previous chunk's output projection is interleaved as PE filler
(o-tiles 0-2 at the first pair, 3 at the second); the final chunk's po
alternates ppo/pj banks so its groups overlap their readbacks. The rope
tables are fetched once (not per chunk); small constants are deferred past
wq on the serialized HWDGE so chunk 0's projections start ~1.5us earlier.
GPSIMD (no PSUM access, no stt opcode) carries SBUF-local q-side work:
rope(q)'s second rotation half, phi(q)'s min and its relu+add assembly.
The y scale (1/denom) is applied on ScalarE (activation Copy with a
per-partition scale pointer), freeing DVE for the PSUM-bound scan ops.
"""

import math
import numpy as np
import ml_dtypes

B, L, HID = 2, 2048, 2048
H, D = 16, 128
HG = 4              # heads per core
C = 128             # chunk size
NCHUNK = L // C     # 16
NK = HID // C       # 16 contraction tiles
NKP = NK // 2       # 8 DoubleRow pair-tiles
EPS = 1e-6
BETA_MIN, BETA_MAX = 0.8, 0.9995
NCORES = 8
GW = HG * D         # 512, per-core projection width
NO = HID // GW      # 4 output col tiles
SCALE = 32.0        # fp8 weight pre-scale
XL_S = 16.0         # x residual pre-scale
E4 = ml_dtypes.float8_e4m3

# fp8 GEMM term counts: 2 = x8@W8 + xl8@W8' (x-quantization compensated),
# 3 = + x8@Wl8 (weight-quantization compensated too). g is ~free (N=4) so it
# keeps 3 terms; q/k/v tolerate W-quant noise (it largely cancels or averages
# out through the normalized attention).
WTERMS = {"q": 1, "k": 1, "v": 3, "g": 3}
PO_SLOTS = ((0, 0), (0, 1), (1, 0))  # (wo term slot, y source: 0=y8 1=ylow)

_CACHE = {}


def _rope_tables():
    half = D // 2
    inv_freq = (1.0 / (10000.0 ** (np.arange(half, dtype=np.float32) /
                                   np.float32(half)))).astype(np.float32)
    t = np.arange(L, dtype=np.float32)
    freqs = t[:, None] * inv_freq[None, :]
    # fold the fp8 weight pre-scale out of q/k here: tables are cos/32, sin/32
    cos = (np.cos(freqs) / SCALE).astype(ml_dtypes.bfloat16)   # [L, 64]
    sin = (np.sin(freqs) / SCALE).astype(ml_dtypes.bfloat16)
    # chunk-major: [128, NCHUNK*64], block c = rows c*128..c*128+128
    def rearr(m):
        return np.ascontiguousarray(
            m.reshape(NCHUNK, C, half).transpose(1, 0, 2).reshape(C, NCHUNK * half))
    return rearr(cos), rearr(sin)


def _build(cfg):
    import concourse.bass as bass
    import concourse.bacc as bacc
    import concourse.tile as tile
    import concourse.mybir as mybir
    from contextlib import ExitStack

    dt = mybir.dt
    F32 = dt.float32
    BF16 = dt.bfloat16
    F8 = dt.float8e4
    DRm = mybir.MatmulPerfMode.DoubleRow
    Alu = mybir.AluOpType
    Act = mybir.ActivationFunctionType
    half = D // 2

    nch = cfg.get("nchunk", NCHUNK)

    nc = bacc.Bacc("TRN2", target_bir_lowering=False, debug=False,
                   enable_asserts=False, num_devices=NCORES)

    # ---- DRAM I/O (host passes PE-blocked layouts, see make_in_maps) ----
    xT_d = nc.dram_tensor("xTb", [NCHUNK, C, 2 * HID], F8, kind="ExternalInput").ap()
    wq_d = nc.dram_tensor("wq3", [C, WTERMS["q"] * NK * GW], F8,
                          kind="ExternalInput").ap()
    wk_d = nc.dram_tensor("wk3", [C, WTERMS["k"] * NK * GW], F8,
                          kind="ExternalInput").ap()
    wv_d = nc.dram_tensor("wv3", [C, WTERMS["v"] * NK * GW], F8,
                          kind="ExternalInput").ap()
    wg_d = nc.dram_tensor("wg3", [C, WTERMS["g"] * NK * HG], F8,
                          kind="ExternalInput").ap()
    wo_d = nc.dram_tensor("wo3", [C, 2 * HG * HID], F8, kind="ExternalInput").ap()
    nbg_d = nc.dram_tensor("nbg4", [C, HG], F32, kind="ExternalInput").ap()
    cos_d = nc.dram_tensor("cosr", [C, NCHUNK * half], BF16, kind="ExternalInput").ap()
    sin_d = nc.dram_tensor("sinr", [C, NCHUNK * half], BF16, kind="ExternalInput").ap()
    mask_d = nc.dram_tensor("maskT", [C, C], F32, kind="ExternalInput").ap()
    id_d = nc.dram_tensor("ident", [C, C], F32, kind="ExternalInput").ap()
    out_d = nc.dram_tensor("out", [L, HID], BF16, kind="ExternalOutput").ap()

    def pair(t, off, step, f):
        b = t[:]
        return bass.AP(tensor=b.tensor, offset=b.offset + off,
                       ap=[b.ap[0], [step, 2], [1, f]])

    with ExitStack() as ctx:
        tc = ctx.enter_context(tile.TileContext(nc))

        cpool = ctx.enter_context(tc.tile_pool(name="consts", bufs=1))
        mask_t = cpool.tile([C, C], F32, tag="mask")
        id_t = cpool.tile([C, C], F32, tag="id")
        id_s = cpool.tile([C, C], BF16, tag="id_s")
        ones_t = cpool.tile([C, C], F32, tag="ones")
        nbg_t = cpool.tile([C, HG], F32, tag="nbg")
        nc.vector.memset(ones_t[:], 1.0)

        with ExitStack() as main:
            wpool = main.enter_context(tc.tile_pool(name="w", bufs=1))
            wq_t = wpool.tile([C, WTERMS["q"] * NK * GW], F8, tag="wq")
            wk_t = wpool.tile([C, WTERMS["k"] * NK * GW], F8, tag="wk")
            wv_t = wpool.tile([C, WTERMS["v"] * NK * GW], F8, tag="wv")
            wg_t = wpool.tile([C, WTERMS["g"] * NK * HG], F8, tag="wg")
            wo_t = wpool.tile([C, 2 * HG * HID], F8, tag="wo")
            nc.sync.dma_start(wg_t[:], wg_d)

            # chunk-local SBUF pools
            xp = main.enter_context(tc.tile_pool(name="xp", bufs=cfg.get("xp", 3)))
            cspool = main.enter_context(tc.tile_pool(name="cspool", bufs=cfg.get("csp", 6)))

            def fetch_x(t, ci, eng=None):
                eng = eng or nc.scalar
                eng.dma_start(t[:, 0:HID], xT_d[ci][:, 0:HID])
                eng.dma_start(t[:, HID:2 * HID], xT_d[ci][:, HID:2 * HID])

            # whole-kernel rope tables fetched once (2 DMAs instead of 32):
            # per-chunk cos/sin are views into these resident tiles
            costab = cspool.tile([C, NCHUNK * half], BF16, tag="costab", bufs=1)
            sintab = cspool.tile([C, NCHUNK * half], BF16, tag="sintab", bufs=1)

            def fetch_cs(ci, eng=None):
                return (costab[:, bass.ts(ci, half)],
                        sintab[:, bass.ts(ci, half)])

            # prefetch chunk 0/1 x + rope tables ahead of the weight
            # stream so chunk 0 isn't queued behind 9 MB of weights
            xpre = []
            cspre = []
            for cpre in range(min(2, nch)):
                t = xp.tile([C, 2 * HID], F8, tag="xtb")
                xpre.append(t)
            fetch_x(xpre[0], 0)
            cspre.append(fetch_cs(0))

            # weights streamed in PE consumption order (g,q,k,v then wo),
            # sliced so the PE can trail the DMA k-pair by k-pair; small
            # constants deferred past wq so chunk 0's q projection isn't
            # queued behind them on the serialized HWDGE
            TW = NK * GW

            def wslices(w_t, w_d, nt):
                for term in range(nt):
                    for hf in range(2):
                        sl = slice(term * TW + hf * TW // 2,
                                   term * TW + (hf + 1) * TW // 2)
                        nc.sync.dma_start(w_t[:, sl], w_d[:, sl])

            wslices(wq_t, wq_d, WTERMS["q"])
            nc.scalar.dma_start(costab[:], cos_d)
            nc.scalar.dma_start(sintab[:], sin_d)
            nc.sync.dma_start(id_t[:], id_d)
            nc.sync.dma_start(nbg_t[:], nbg_d)
            nc.scalar.copy(id_s[:], id_t[:])
            cspre.append(fetch_cs(1))
            wslices(wk_t, wk_d, WTERMS["k"])
            fetch_x(xpre[1], 1)
            nc.sync.dma_start(mask_t[:], mask_d)
            wslices(wv_t, wv_d, WTERMS["v"])
            for term in (0, 1):
                ts_ = bass.ts(term, HG * HID)
                nc.sync.dma_start(wo_t[:, ts_], wo_d[:, ts_])
            big2 = main.enter_context(tc.tile_pool(name="big2", bufs=cfg.get("big2", 3)))
            sml = main.enter_context(tc.tile_pool(name="sml", bufs=cfg.get("sml", 4)))
            spool = main.enter_context(tc.tile_pool(name="spool", bufs=2))
            ypool = main.enter_context(tc.tile_pool(name="ypool", bufs=2))
            osb = main.enter_context(tc.tile_pool(name="osb", bufs=cfg.get("osb", 5)))

            # psum pools: pj 2 + ppo 2 + ptp 2 + pnu 2 = 8 banks
            pj = main.enter_context(tc.tile_pool(
                name="pj", bufs=cfg.get("pj", 2), space="PSUM"))
            ppo = main.enter_context(tc.tile_pool(
                name="ppo", bufs=cfg.get("ppo", 1), space="PSUM"))
            ptp = main.enter_context(tc.tile_pool(
                name="ptp", bufs=cfg.get("ptp", 2), space="PSUM"))
            pnu = main.enter_context(tc.tile_pool(
                name="pnu", bufs=cfg.get("pnu", 2), space="PSUM"))
            pbeta = main.enter_context(tc.tile_pool(
                name="pbeta", bufs=cfg.get("pbeta", 1), space="PSUM"))

            S_cur = []
            for h in range(HG):
                s0 = spool.tile([C, D + 1], BF16, tag=f"s{h}")
                nc.vector.memset(s0[:], 0.0)
                S_cur.append(s0)

            def proj_mms(ps, fw, w_t, tw, nterm, xtb):
                n = 0
                for term in range(nterm):
                    xoff = HID if term == 1 else 0
                    for kp in range(NKP):
                        nc.tensor.matmul(
                            ps[:, 0:fw],
                            pair(xtb, xoff + kp * 2 * C, C, C),
                            pair(w_t, term * tw + kp * 2 * fw, fw, fw),
                            start=(n == 0), stop=(n == nterm * NKP - 1),
                            perf_mode=DRm)
                        n += 1

            def rope(src, dst, tmp, cs, tmp2=None):
                # tmp2 set: de-half on DVE, do-half on GPSIMD concurrently
                cos_c, sin_c = cs
                ed = nc.vector
                eo = nc.gpsimd if tmp2 is not None else nc.vector
                se = src[:].rearrange("p (h d) -> p h d", h=HG)[:, :, 0:half]
                so = src[:].rearrange("p (h d) -> p h d", h=HG)[:, :, half:D]
                de = dst[:].rearrange("p (h d) -> p h d", h=HG)[:, :, 0:half]
                do = dst[:].rearrange("p (h d) -> p h d", h=HG)[:, :, half:D]
                cc = bass.AP(tensor=cos_c.tensor, offset=cos_c.offset,
                             ap=[cos_c.ap[0], [0, HG], [1, half]])
                ss = bass.AP(tensor=sin_c.tensor, offset=sin_c.offset,
                             ap=[sin_c.ap[0], [0, HG], [1, half]])
                t1 = tmp[:].rearrange("p (h d) -> p h d", h=HG)[:, :, 0:half]
                t2 = tmp[:].rearrange("p (h d) -> p h d", h=HG)[:, :, half:D]
                tb = tmp2 if tmp2 is not None else tmp
                t3 = tb[:].rearrange("p (h d) -> p h d", h=HG)[:, :, 0:half]
                t4 = tb[:].rearrange("p (h d) -> p h d", h=HG)[:, :, half:D]
                ed.tensor_tensor(out=t1, in0=se, in1=cc, op=Alu.mult)
                ed.tensor_tensor(out=t2, in0=so, in1=ss, op=Alu.mult)
                ed.tensor_tensor(out=de, in0=t1, in1=t2, op=Alu.subtract)
                eo.tensor_tensor(out=t3, in0=se, in1=ss, op=Alu.mult)
                eo.tensor_tensor(out=t4, in0=so, in1=cc, op=Alu.mult)
                eo.tensor_tensor(out=do, in0=t3, in1=t4, op=Alu.add)

            po_prev = None  # (yt8, ytl) of previous chunk

            def emit_po(ysrcs, c_out, orange=None, final=False):
                yt8_, ytl_ = ysrcs
                nmm = 2 * len(PO_SLOTS)
                for o in (orange if orange is not None else range(NO)):
                    # in the epilogue the projection banks are idle: alternate
                    # pools so o-tile groups overlap their readback copies
                    if final and o % 2 == 1:
                        out_ps = pj.tile([C, GW], F32, tag="big")
                    else:
                        out_ps = ppo.tile([C, GW], F32, tag="po")
                    n = 0
                    # hp-outer: the first MMs only need heads 0-1, so the
                    # group can start before heads 2-3 finish
                    for hp in range(HG // 2):
                        for slot, ysel in PO_SLOTS:
                            ysrc = yt8_ if ysel == 0 else ytl_
                            nc.tensor.matmul(
                                out_ps[:],
                                pair(ysrc, hp * 2 * C, C, C),
                                pair(wo_t,
                                     slot * HG * HID + (2 * hp) * HID + o * GW,
                                     HID, GW),
                                start=(n == 0), stop=(n == nmm - 1),
                                perf_mode=DRm)
                            n += 1
                    out_sb = osb.tile([C, GW], BF16, tag="osb")
                    if o % 2 == 0:
                        nc.scalar.mul(out_sb[:], out_ps[:], 1.0 / SCALE)
                    else:
                        nc.vector.tensor_scalar_mul(out_sb[:], out_ps[:],
                                                    1.0 / SCALE)
                    nc.sync.dma_start(out_d[bass.ts(c_out, C), bass.ts(o, GW)],
                                      out_sb[:])

            def proj_phase(c):
                """Projections + beta chain + rope/phi/khat for chunk c.
                Emitted two chunks ahead of the scan so the elementwise
                chains overlap earlier chunks' scans."""
                if c < len(xpre):
                    xtb = xpre[c]
                    cos_c, sin_c = cspre[c]
                else:
                    xtb = xp.tile([C, 2 * HID], F8, tag="xtb")
                    fetch_x(xtb, c)
                    cos_c, sin_c = fetch_cs(c)

                # ---- projections (PE) interleaved with beta chain ----
                # g first so the long beta dependency chain starts early
                g_ps = pj.tile([C, GW], F32, tag="big")
                proj_mms(g_ps, HG, wg_t, NK * HG, WTERMS["g"], xtb)
                beta_sb = sml.tile([C, HG], F32, tag="beta")
                nc.scalar.activation(beta_sb[:], g_ps[:, 0:HG], Act.Exp,
                                     scale=-1.0 / SCALE)
                nc.vector.scalar_tensor_tensor(
                    out=beta_sb[:], in0=beta_sb[:], scalar=1.0,
                    in1=nbg_t[:], op0=Alu.mult, op1=Alu.mult)
                nc.vector.tensor_scalar_add(beta_sb[:], beta_sb[:], 1.0)
                nc.vector.reciprocal(beta_sb[:], beta_sb[:])
                nc.gpsimd.tensor_scalar(out=beta_sb[:], in0=beta_sb[:],
                                        scalar1=BETA_MIN, scalar2=BETA_MAX,
                                        op0=Alu.max, op1=Alu.min)

                q_ps = pj.tile([C, GW], F32, tag="big")
                proj_mms(q_ps, GW, wq_t, TW, WTERMS["q"], xtb)
                q_sb = big2.tile([C, GW], BF16, tag="q")
                nc.scalar.copy(q_sb[:], q_ps[:])

                # beta transposes ride the pnu rotation between head uses
                btp_ps = pbeta.tile([C, C], F32, tag="bt")
                nc.tensor.transpose(btp_ps[0:HG, 0:C], beta_sb[:], id_t[:])
                btp_sb = sml.tile([HG, C], F32, tag="btp")
                nc.scalar.copy(btp_sb[:], btp_ps[0:HG, 0:C])
                aT_sb = sml.tile([HG, C], F32, tag="aT")
                nc.vector.tensor_tensor_scan(
                    out=aT_sb[:], data0=btp_sb[:], data1=ones_t[0:HG, :],
                    initial=1.0, op0=Alu.mult, op1=Alu.mult)

                k_ps = pj.tile([C, GW], F32, tag="big")
                proj_mms(k_ps, GW, wk_t, TW, WTERMS["k"], xtb)
                k_sb = big2.tile([C, GW], BF16, tag="k")
                nc.scalar.copy(k_sb[:], k_ps[:])

                a_ps = pbeta.tile([C, C], F32, tag="bt")
                nc.tensor.transpose(a_ps[:, 0:HG], aT_sb[:], id_t[0:HG, 0:HG])
                a_sb = sml.tile([C, HG], F32, tag="a")
                nc.scalar.copy(a_sb[:], a_ps[:, 0:HG])
                ainv_sb = sml.tile([C, HG], F32, tag="ainv")
                nc.vector.reciprocal(ainv_sb[:], a_sb[:])
                diag4 = sml.tile([HG, HG], F32, tag="diag4")
                nc.vector.tensor_scalar(out=diag4[:], in0=id_t[0:HG, 0:HG],
                                        scalar1=aT_sb[:, C - 1:C], scalar2=None,
                                        op0=Alu.mult)

                # rope(q) early on DVE; phi(q) min + assembly on GPSIMD
                # (the q path is off the serial S/z recurrence, so the slow
                # Pool engine can carry it)
                qr = big2.tile([C, GW], BF16, tag="qr")
                rtq = big2.tile([C, GW], BF16, tag="rtq")
                rtq2 = big2.tile([C, GW], BF16, tag="rtq2")
                rope(q_sb, qr, rtq, (cos_c, sin_c), rtq2)
                tmq = big2.tile([C, GW], BF16, tag="mq")
                nc.vector.tensor_scalar_min(tmq[:], qr[:], 0.0)
                teq = big2.tile([C, GW], BF16, tag="eq")
                nc.scalar.activation(teq[:], tmq[:], Act.Exp)
                # phi(q) assembled on GPSIMD (no stt opcode there: relu + add)
                rlq = big2.tile([C, GW], BF16, tag="rlq")
                nc.gpsimd.tensor_scalar_max(rlq[:], qr[:], 0.0)
                phiq = big2.tile([C, GW], BF16, tag="phq", bufs=4)
                nc.gpsimd.tensor_tensor(out=phiq[:], in0=rlq[:], in1=teq[:],
                                        op=Alu.add)

                v_ps = pj.tile([C, GW], F32, tag="big")
                proj_mms(v_ps, GW, wv_t, TW, WTERMS["v"], xtb)
                v_sb = big2.tile([C, HG * (D + 1)], BF16, tag="v", bufs=4)
                v_aug = v_sb[:].rearrange("p (h e) -> p h e", e=D + 1)
                nc.scalar.copy(v_aug[:, :, 0:D],
                               v_ps[:, 0:GW].rearrange("p (h e) -> p h e", e=D))
                nc.vector.memset(v_aug[:, :, D:D + 1], SCALE)

                acb_ps = pbeta.tile([C, C], F32, tag="bt")
                nc.tensor.matmul(acb_ps[:, 0:HG], ones_t[0:HG, :], diag4[:],
                                 start=True, stop=True)
                acb_sb = sml.tile([C, HG], F32, tag="acb")
                nc.scalar.copy(acb_sb[:], acb_ps[:, 0:HG])
                acdiv_sb = sml.tile([C, HG], F32, tag="acdiv")
                nc.vector.tensor_tensor(out=acdiv_sb[:], in0=ainv_sb[:],
                                        in1=acb_sb[:], op=Alu.mult)

                # rope(k) + phi(k) stay on the fast engines: the k-path feeds
                # the serial S/z recurrence (khat -> U -> S_new)
                kr = big2.tile([C, GW], BF16, tag="kr")
                rtk = big2.tile([C, GW], BF16, tag="rtk")
                rope(k_sb, kr, rtk, (cos_c, sin_c))
                tmk = big2.tile([C, GW], BF16, tag="mk")
                nc.vector.tensor_scalar_min(tmk[:], kr[:], 0.0)
                tek = big2.tile([C, GW], BF16, tag="ek")
                nc.scalar.activation(tek[:], tmk[:], Act.Exp)
                phik = big2.tile([C, GW], BF16, tag="phk", bufs=4)
                nc.vector.scalar_tensor_tensor(out=phik[:], in0=kr[:],
                                               scalar=0.0, in1=tek[:],
                                               op0=Alu.max, op1=Alu.add)

                # khat for all heads, hoisted off the per-head critical path
                khats = []
                for h in range(HG):
                    khat = sml.tile([C, D], BF16, tag="khat", bufs=12)
                    eng = nc.gpsimd if h < 3 else nc.vector
                    eng.tensor_scalar_mul(khat[:], phik[:, bass.ts(h, D)],
                                          acdiv_sb[:, h:h + 1])
                    khats.append(khat[:])

                return dict(phiq=phiq, phik=phik, khats=khats, v_sb=v_sb,
                            ainv_sb=ainv_sb, acb_sb=acb_sb)

            def scan_phase(c, P, last=False):
                nonlocal po_prev
                phiq, phik = P["phiq"], P["phik"]
                khats, v_sb = P["khats"], P["v_sb"]
                ainv_sb, acb_sb = P["ainv_sb"], P["acb_sb"]

                # ---- scan, 2-wide head pipeline ----
                use_ytl = any(ysel == 1 for _, ysel in PO_SLOTS)
                yt8 = ypool.tile([C, HG * C], F8, tag="yt8")
                if use_ytl:
                    ytl = ypool.tile([C, HG * C], F8, tag="ytl")
                else:
                    ytl = None

                def pair_tp(p):
                    # both heads' q/k transposes land in one PSUM tile so a
                    # single wide DVE copy moves them to SBUF (2x bf16 mode)
                    tp6 = ptp.tile([C, 6 * D], BF16, tag="tp6")
                    for i, h in enumerate((p, p + 1)):
                        hs = bass.ts(h, D)
                        nc.tensor.transpose(tp6[:, (2 * i) * D:(2 * i + 1) * D],
                                            phiq[:, hs], id_s[:])
                        nc.tensor.transpose(tp6[:, (2 * i + 1) * D:(2 * i + 2) * D],
                                            phik[:, hs], id_s[:])
                    qkT = sml.tile([C, 4 * D], BF16, tag="qkT")
                    nc.vector.tensor_copy(qkT[:], tp6[:, 0:4 * D])
                    return tp6, qkT

                def head_A(h, i, st):
                    tp6, qkT = st
                    nuA = pnu.tile([C, 3 * (D + 1) - 1], F32, tag="nuA")
                    Ar = nuA[:, 2 * (D + 1):3 * (D + 1) - 1]
                    nc.tensor.matmul(Ar[:], qkT[:, (2 * i + 1) * D:(2 * i + 2) * D],
                                     qkT[:, (2 * i) * D:(2 * i + 1) * D],
                                     start=True, stop=True)
                    A_sb = sml.tile([C, C], BF16, tag="A")
                    nc.vector.scalar_tensor_tensor(
                        out=A_sb[:], in0=Ar[:],
                        scalar=ainv_sb[:, h:h + 1], in1=mask_t[:],
                        op0=Alu.mult, op1=Alu.mult)
                    return (*st, nuA, A_sb)

                def head_nu(h, i, st):
                    tp6, qkT, nuA, A_sb = st
                    nu = nuA[:, 0:D + 1]
                    U = nuA[:, D + 1:2 * (D + 1)]
                    vh = v_sb[:, h * (D + 1):(h + 1) * (D + 1)]
                    # the A_sb-consuming matmul opens the bank's accumulation
                    # group: it naturally orders after the DVE mask, so the
                    # Ar readback completes before the bank is re-zeroed
                    nc.tensor.matmul(nu[:], A_sb[:], vh, start=True, stop=False)
                    nc.tensor.matmul(U[:], khats[h], vh, start=False,
                                     stop=False)
                    nc.tensor.matmul(nu[:], qkT[:, (2 * i) * D:(2 * i + 1) * D],
                                     S_cur[h][:], start=False, stop=True)
                    # recip/ybf first: the y-transpose unblocks before the
                    # (slack-tolerant) S update; denom = phi_q . z is strictly
                    # positive (phi > 0) so the reference's +eps (~1e-8
                    # relative) is dropped and the reciprocal reads PSUM
                    rd = sml.tile([C, 1], F32, tag="rd")
                    nc.vector.reciprocal(rd[:], nu[:, D:D + 1])
                    y_bf = sml.tile([C, D], BF16, tag="ybf")
                    nc.scalar.activation(y_bf[:], nu[:, 0:D], Act.Copy,
                                         scale=rd[:])
                    S_new = spool.tile([C, D + 1], BF16, tag=f"s{h}")
                    nc.vector.scalar_tensor_tensor(
                        out=S_new[:], in0=S_cur[h][:], scalar=acb_sb[:, h:h + 1],
                        in1=U, op0=Alu.mult, op1=Alu.add)
                    S_cur[h] = S_new
                    return (*st, y_bf)

                def head_yT(h, i, st):
                    tp6, y_bf = st[0], st[-1]
                    nc.tensor.transpose(tp6[:, (4 + i) * D:(5 + i) * D],
                                        y_bf[:], id_s[:])

                # out-projection of the PREVIOUS chunk is interleaved into the
                # scan as PE filler work behind the DVE/ACT dependency chains
                for p in (0, 2):
                    st = pair_tp(p)
                    s0 = head_A(p, 0, st)
                    s1 = head_A(p + 1, 1, st)
                    s0 = head_nu(p, 0, s0)
                    s1 = head_nu(p + 1, 1, s1)
                    if po_prev is not None:
                        emit_po(po_prev[0], po_prev[1],
                                orange=(0, 1, 2) if p == 0 else (3,))
                    head_yT(p, 0, s0)
                    head_yT(p + 1, 1, s1)
                    tp6 = st[0]
                    ys = slice(p * C, (p + 2) * C)
                    nc.scalar.copy(yt8[:, ys], tp6[:, 4 * D:6 * D])
                    if use_ytl:
                        nc.vector.tensor_tensor(out=ytl[:, ys],
                                                in0=tp6[:, 4 * D:6 * D],
                                                in1=yt8[:, ys], op=Alu.subtract)
                po_prev = ((yt8, ytl), c)

            # ---- chunk pipeline: projections run two chunks ahead of the
            # scan so their elementwise chains overlap earlier scans ----
            pend = [proj_phase(0)]
            if nch > 1:
                pend.append(proj_phase(1))
            for c in range(nch):
                scan_phase(c, pend.pop(0), last=(c == nch - 1))
                if c + 2 < nch:
                    pend.append(proj_phase(c + 2))

            emit_po(po_prev[0], po_prev[1], final=True)

    nc.compile()
    return nc


def _get_nc(cfg_key="default", **cfg):
    if cfg_key not in _CACHE:
        _CACHE[cfg_key] = _build(cfg)
    return _CACHE[cfg_key]


def _blk(m, fw):
    # [HID, fw] -> [C, NK*fw] with block k = m[k*128:(k+1)*128, :]
    return np.ascontiguousarray(
        m.reshape(NK, C, fw).transpose(1, 0, 2).reshape(C, NK * fw))


def _w3(W, fw, nterm):
    """W [HID, fw] f32 -> [C, nterm*NK*fw] e4m3: fp8(32W) | fp8(2W) | fp8(32Wl)."""
    W = np.asarray(W, np.float32)
    t0 = (SCALE * W).astype(E4)
    terms = [_blk(t0, fw), _blk((2.0 * W).astype(E4), fw)]
    if nterm == 3:
        wl = W - t0.astype(np.float32) / SCALE
        terms.append(_blk((SCALE * wl).astype(E4), fw))
    return np.ascontiguousarray(np.concatenate(terms[:nterm], axis=1))


def _wo3(Wo):
    """Wo [GW, HID] f32 -> [C, 2*HG*HID] e4m3, blocked by head: fp8(32Wo)
    (shared by the y8 and y-residual terms, both at 32x scale) | fp8(32*Wol)."""
    Wo = np.asarray(Wo, np.float32)
    t0 = (SCALE * Wo).astype(E4)
    wl = Wo - t0.astype(np.float32) / SCALE
    t2 = (SCALE * wl).astype(E4)

    def blk(m):
        return m.reshape(HG, C, HID).transpose(1, 0, 2).reshape(C, HG * HID)

    return np.ascontiguousarray(np.concatenate([blk(t0), blk(t2)], axis=1))


def make_in_maps(x, Wq, Wk, Wv, Wg, bg, Wo, bo):
    cosr, sinr = _rope_tables()
    maskT = np.triu(np.ones((C, C), np.float32))
    ident = np.eye(C, dtype=np.float32)
    x = np.asarray(x, np.float32)
    Wq, Wk, Wv = np.asarray(Wq), np.asarray(Wk), np.asarray(Wv)
    Wg, bg, Wo = np.asarray(Wg), np.asarray(bg), np.asarray(Wo)
    in_maps = []
    xTb_cache = {}
    for core in range(NCORES):
        b, hg = divmod(core, 4)
        cs = slice(hg * GW, (hg + 1) * GW)
        hsl = slice(hg * HG, (hg + 1) * HG)
        if b not in xTb_cache:
            # xTb[c, p, k*128+f] = x[b][c*128+f, k*128+p]; fp8 + 16*residual
            xT = np.ascontiguousarray(
                x[b].reshape(NCHUNK, C, NK, C).transpose(0, 3, 2, 1)
                .reshape(NCHUNK, C, HID)).astype(np.float32)
            x8 = xT.astype(E4)
            xl8 = (XL_S * (xT - x8.astype(np.float32))).astype(E4)
            xTb_cache[b] = np.ascontiguousarray(
                np.concatenate([x8, xl8], axis=2))
        in_maps.append({
            "xTb": xTb_cache[b],
            "wq3": _w3(Wq[:, cs], GW, WTERMS["q"]),
            "wk3": _w3(Wk[:, cs], GW, WTERMS["k"]),
            "wv3": _w3(Wv[:, cs], GW, WTERMS["v"]),
            "wg3": _w3(Wg[:, hsl], HG, WTERMS["g"]),
            "wo3": _wo3(Wo[cs, :]),
            "nbg4": np.tile(np.exp(-bg[None, hsl]), (C, 1)).astype(np.float32),
            "cosr": cosr, "sinr": sinr,
            "maskT": maskT, "ident": ident,
        })
    return in_maps


def kernel(x, Wq, Wk, Wv, Wg, bg, Wo, bo, _trace=False, **cfg):
    from concourse.bass_utils import run_bass_kernel_spmd
    nc = _get_nc(**cfg)
    in_maps = make_in_maps(x, Wq, Wk, Wv, Wg, bg, Wo, bo)
    res = run_bass_kernel_spmd(nc, in_maps, core_ids=list(range(NCORES)),
                               trace=_trace)
    out = np.zeros((B, L, HID), np.float32)
    for core in range(NCORES):
        b = core // 4
        out[b] += res.results[core]["out"].astype(np.float32)
    out += np.asarray(bo, np.float32)[None, None, :]
    kernel._last_results = res
    return out



# revision 74
# speedup vs baseline: 1.0238x; 1.0009x over previous
"""Trainium2 Bass kernel for nn_DeltaNet_22488448762128 (v4).

Full-input contract: kernel(**inputs) takes the unsharded numpy inputs and
returns the full [B, L, HID] output. Internally shards across 8 NeuronCores:
core = (b, hg) with b in {0,1} and hg in {0..3} head-groups of 4 heads.
Each core computes projections for its 4 heads, a chunked (C=128) linear
attention scan, and a partial output projection; the host sums the 4 partial
outputs per batch element and adds bo.

Math (per head, chunk c of size C, state S aug with z column):
  a_t   = cumprod(beta) within chunk;  aC = a_{C-1}
  q~_t  = phi(rope(q))_t * a_t ;  k^_s = phi(rope(k))_s * aC / a_s
  A^T[s,t] = (phi_k_s . q~_t) * (1/a_s) * [s<=t]
  nu    = A^T.T @ [V|1] + q~ @ S_aug   ;  y_t = nu[:, :D] / (nu[:, D] + eps)
  S_aug = aC * S_aug + k^T @ [V|1]

Precision: projections and the output projection run on the PE in fp8 (e4m3)
DoubleRow mode (pairs two 128-row contraction tiles per pass) with
error-compensated term stacks
  x @ W  =  x8 @ W8  [+ xl8 @ W8']  [+ x8 @ Wl8]
(x8 = fp8(x), xl8 = fp8(16*(x - x8)), Wl = W - fp8(32W)/32). Measured
per-path sensitivity sets the term counts (WTERMS): q/k run 1-term (their
quantization noise cancels through the normalized attention), v and the
output projection need all terms (their noise reaches y linearly), g is
free. Weights are pre-scaled by 32 into e4m3's normal range; the inverse
scale is folded into the rope tables, the sigmoid's activation scale, the
augmented-ones column (=32), and the final output-copy scale, so unscaling
costs zero extra instructions. The reference's +eps (1e-6) on the strictly
positive denominator is dropped. The scan runs in bf16 operands with fp32
PSUM accumulation; the rope/phi elementwise path is bf16 end-to-end (2-4x
DVE modes). Output is returned bf16; the host sums partials in fp32.

PSUM banks (8): pj[big x2] g/q/k/v projection groups (+ final-po overflow);
ppo[po x1] output projection; ptp[tp6 x2] per-head-PAIR q/k/y transposes
(one wide 2x-mode DVE copy per pair instead of per-head ACT copies);
pnu[nuA x2] per-head A|nu|U groups (A's group is closed before the nu/U
group reopens the bank; the reopening matmul consumes A_sb, so it naturally
orders after the DVE mask's Ar readback); pbeta[bt x1] the beta-chain
transposes, decoupled from the scan's pnu ring so the two-chunk-ahead
projection phase can't stall the scan.

Schedule: two-level software pipeline. Outer: proj_phase(c+2) (projections,
beta chain, rope/phi, khat) is emitted two chunks ahead of scan_phase(c),
so the long rope->phi elementwise chains overlap earlier chunks' scans and
the PE never waits on them in steady state. Inner: 2-wide pipelined head
pairs; the previous chunk's output projection is interleaved as PE filler
(o-tiles 0-2 at the first pair, 3 at the second); the final chunk's po
alternates ppo/pj banks so its groups overlap their readbacks. The rope
tables are fetched once (not per chunk); small constants are deferred past
wq on the serialized HWDGE so chunk 0's projections start ~1.5us earlier.
GPSIMD (no PSUM access, no stt opcode) carries SBUF-local q-side work:
rope(q)'s second rotation half, phi(q)'s min and its relu+add assembly.
The y scale (1/denom) is applied on ScalarE (activation Copy with a
per-partition scale pointer), freeing DVE for the PSUM-bound scan ops.
"""

import math
import numpy as np
import ml_dtypes

B, L, HID = 2, 2048, 2048
H, D = 16, 128
HG = 4              # heads per core
C = 128             # chunk size
NCHUNK = L // C     # 16
NK = HID // C       # 16 contraction tiles
NKP = NK // 2       # 8 DoubleRow pair-tiles
EPS = 1e-6
BETA_MIN, BETA_MAX = 0.8, 0.9995
NCORES = 8
GW = HG * D         # 512, per-core projection width
NO = HID // GW      # 4 output col tiles
SCALE = 32.0        # fp8 weight pre-scale
XL_S = 16.0         # x residual pre-scale
E4 = ml_dtypes.float8_e4m3

# fp8 GEMM term counts: 2 = x8@W8 + xl8@W8' (x-quantization compensated),
# 3 = + x8@Wl8 (weight-quantization compensated too). g is ~free (N=4) so it
# keeps 3 terms; q/k/v tolerate W-quant noise (it largely cancels or averages
# out through the normalized attention).
WTERMS = {"q": 1, "k": 1, "v": 3, "g": 3}
PO_SLOTS = ((0, 0), (0, 1), (1, 0))  # (wo term slot, y source: 0=y8 1=ylow)

_CACHE = {}


def _rope_tables():
    half = D // 2
    inv_freq = (1.0 / (10000.0 ** (np.arange(half, dtype=np.float32) /
                                   np.float32(half)))).astype(np.float32)
    t = np.arange(L, dtype=np.float32)
    freqs = t[:, None] * inv_freq[None, :]
    # fold the fp8 weight pre-scale out of q/k here: tables are cos/32, sin/32
    cos = (np.cos(freqs) / SCALE).astype(ml_dtypes.bfloat16)   # [L, 64]
    sin = (np.sin(freqs) / SCALE).astype(ml_dtypes.bfloat16)
    # chunk-major: [128, NCHUNK*64], block c = rows c*128..c*128+128
    def rearr(m):
        return np.ascontiguousarray(
            m.reshape(NCHUNK, C, half).transpose(1, 0, 2).reshape(C, NCHUNK * half))
    return rearr(cos), rearr(sin)


def _build(cfg):
    import concourse.bass as bass
    import concourse.bacc as bacc
    import concourse.tile as tile
    import concourse.mybir as mybir
    from contextlib import ExitStack

    dt = mybir.dt
    F32 = dt.float32
    BF16 = dt.bfloat16
    F8 = dt.float8e4
    DRm = mybir.MatmulPerfMode.DoubleRow
    Alu = mybir.AluOpType
    Act = mybir.ActivationFunctionType
    half = D // 2

    nch = cfg.get("nchunk", NCHUNK)

    nc = bacc.Bacc("TRN2", target_bir_lowering=False, debug=False,
                   enable_asserts=False, num_devices=NCORES)

    # ---- DRAM I/O (host passes PE-blocked layouts, see make_in_maps) ----
    xT_d = nc.dram_tensor("xTb", [NCHUNK, C, 2 * HID], F8, kind="ExternalInput").ap()
    wq_d = nc.dram_tensor("wq3", [C, WTERMS["q"] * NK * GW], F8,
                          kind="ExternalInput").ap()
    wk_d = nc.dram_tensor("wk3", [C, WTERMS["k"] * NK * GW], F8,
                          kind="ExternalInput").ap()
    wv_d = nc.dram_tensor("wv3", [C, WTERMS["v"] * NK * GW], F8,
                          kind="ExternalInput").ap()
    wg_d = nc.dram_tensor("wg3", [C, WTERMS["g"] * NK * HG], F8,
                          kind="ExternalInput").ap()
    wo_d = nc.dram_tensor("wo3", [C, 2 * HG * HID], F8, kind="ExternalInput").ap()
    nbg_d = nc.dram_tensor("nbg4", [C, HG], F32, kind="ExternalInput").ap()
    cos_d = nc.dram_tensor("cosr", [C, NCHUNK * half], BF16, kind="ExternalInput").ap()
    sin_d = nc.dram_tensor("sinr", [C, NCHUNK * half], BF16, kind="ExternalInput").ap()
    mask_d = nc.dram_tensor("maskT", [C, C], F32, kind="ExternalInput").ap()
    id_d = nc.dram_tensor("ident", [C, C], F32, kind="ExternalInput").ap()
    out_d = nc.dram_tensor("out", [L, HID], BF16, kind="ExternalOutput").ap()

    def pair(t, off, step, f):
        b = t[:]
        return bass.AP(tensor=b.tensor, offset=b.offset + off,
                       ap=[b.ap[0], [step, 2], [1, f]])

    with ExitStack() as ctx:
        tc = ctx.enter_context(tile.TileContext(nc))

        cpool = ctx.enter_context(tc.tile_pool(name="consts", bufs=1))
        mask_t = cpool.tile([C, C], F32, tag="mask")
        id_t = cpool.tile([C, C], F32, tag="id")
        id_s = cpool.tile([C, C], BF16, tag="id_s")
        ones_t = cpool.tile([C, C], F32, tag="ones")
        nbg_t = cpool.tile([C, HG], F32, tag="nbg")
        nc.vector.memset(ones_t[:], 1.0)

        with ExitStack() as main:
            wpool = main.enter_context(tc.tile_pool(name="w", bufs=1))
            wq_t = wpool.tile([C, WTERMS["q"] * NK * GW], F8, tag="wq")
            wk_t = wpool.tile([C, WTERMS["k"] * NK * GW], F8, tag="wk")
            wv_t = wpool.tile([C, WTERMS["v"] * NK * GW], F8, tag="wv")
            wg_t = wpool.tile([C, WTERMS["g"] * NK * HG], F8, tag="wg")
            wo_t = wpool.tile([C, 2 * HG * HID], F8, tag="wo")
            nc.sync.dma_start(wg_t[:], wg_d)

            # chunk-local SBUF pools
            xp = main.enter_context(tc.tile_pool(name="xp", bufs=cfg.get("xp", 3)))
            cspool = main.enter_context(tc.tile_pool(name="cspool", bufs=cfg.get("csp", 6)))

            def fetch_x(t, ci, eng=None):
                eng = eng or nc.scalar
                eng.dma_start(t[:, 0:HID], xT_d[ci][:, 0:HID])
                eng.dma_start(t[:, HID:2 * HID], xT_d[ci][:, HID:2 * HID])

            # whole-kernel rope tables fetched once (2 DMAs instead of 32):
            # per-chunk cos/sin are views into these resident tiles
            costab = cspool.tile([C, NCHUNK * half], BF16, tag="costab", bufs=1)
            sintab = cspool.tile([C, NCHUNK * half], BF16, tag="sintab", bufs=1)

            def fetch_cs(ci, eng=None):
                return (costab[:, bass.ts(ci, half)],
                        sintab[:, bass.ts(ci, half)])

            # prefetch chunk 0/1 x + rope tables ahead of the weight
            # stream so chunk 0 isn't queued behind 9 MB of weights
            xpre = []
            cspre = []
            for cpre in range(min(2, nch)):
                t = xp.tile([C, 2 * HID], F8, tag="xtb")
                xpre.append(t)
            fetch_x(xpre[0], 0)
            cspre.append(fetch_cs(0))

            # weights streamed in PE consumption order (g,q,k,v then wo),
            # sliced so the PE can trail the DMA k-pair by k-pair; small
            # constants deferred past wq so chunk 0's q projection isn't
            # queued behind them on the serialized HWDGE
            TW = NK * GW

            def wslices(w_t, w_d, nt):
                for term in range(nt):
                    for hf in range(2):
                        sl = slice(term * TW + hf * TW // 2,
                                   term * TW + (hf + 1) * TW // 2)
                        nc.sync.dma_start(w_t[:, sl], w_d[:, sl])

            wslices(wq_t, wq_d, WTERMS["q"])
            nc.scalar.dma_start(costab[:], cos_d)
            nc.scalar.dma_start(sintab[:], sin_d)
            nc.sync.dma_start(id_t[:], id_d)
            nc.sync.dma_start(nbg_t[:], nbg_d)
            nc.scalar.copy(id_s[:], id_t[:])
            cspre.append(fetch_cs(1))
            wslices(wk_t, wk_d, WTERMS["k"])
            fetch_x(xpre[1], 1)
            nc.sync.dma_start(mask_t[:], mask_d)
            wslices(wv_t, wv_d, WTERMS["v"])
            for term in (0, 1):
                ts_ = bass.ts(term, HG * HID)
                nc.sync.dma_start(wo_t[:, ts_], wo_d[:, ts_])
            big2 = main.enter_context(tc.tile_pool(name="big2", bufs=cfg.get("big2", 3)))
            sml = main.enter_context(tc.tile_pool(name="sml", bufs=cfg.get("sml", 4)))
            spool = main.enter_context(tc.tile_pool(name="spool", bufs=2))
            ypool = main.enter_context(tc.tile_pool(name="ypool", bufs=2))
            osb = main.enter_context(tc.tile_pool(name="osb", bufs=cfg.get("osb", 5)))

            # psum pools: pj 2 + ppo 2 + ptp 2 + pnu 2 = 8 banks
            pj = main.enter_context(tc.tile_pool(
                name="pj", bufs=cfg.get("pj", 2), space="PSUM"))
            ppo = main.enter_context(tc.tile_pool(
                name="ppo", bufs=cfg.get("ppo", 1), space="PSUM"))
            ptp = main.enter_context(tc.tile_pool(
                name="ptp", bufs=cfg.get("ptp", 2), space="PSUM"))
            pnu = main.enter_context(tc.tile_pool(
                name="pnu", bufs=cfg.get("pnu", 2), space="PSUM"))
            pbeta = main.enter_context(tc.tile_pool(
                name="pbeta", bufs=cfg.get("pbeta", 1), space="PSUM"))

            S_cur = []
            for h in range(HG):
                s0 = spool.tile([C, D + 1], BF16, tag=f"s{h}")
                nc.vector.memset(s0[:], 0.0)
                S_cur.append(s0)

            def proj_mms(ps, fw, w_t, tw, nterm, xtb):
                n = 0
                for term in range(nterm):
                    xoff = HID if term == 1 else 0
                    for kp in range(NKP):
                        nc.tensor.matmul(
                            ps[:, 0:fw],
                            pair(xtb, xoff + kp * 2 * C, C, C),
                            pair(w_t, term * tw + kp * 2 * fw, fw, fw),
                            start=(n == 0), stop=(n == nterm * NKP - 1),
                            perf_mode=DRm)
                        n += 1

            def rope(src, dst, tmp, cs, tmp2=None):
                # tmp2 set: de-half on DVE, do-half on GPSIMD concurrently
                cos_c, sin_c = cs
                ed = nc.vector
                eo = nc.gpsimd if tmp2 is not None else nc.vector
                se = src[:].rearrange("p (h d) -> p h d", h=HG)[:, :, 0:half]
                so = src[:].rearrange("p (h d) -> p h d", h=HG)[:, :, half:D]
                de = dst[:].rearrange("p (h d) -> p h d", h=HG)[:, :, 0:half]
                do = dst[:].rearrange("p (h d) -> p h d", h=HG)[:, :, half:D]
                cc = bass.AP(tensor=cos_c.tensor, offset=cos_c.offset,
                             ap=[cos_c.ap[0], [0, HG], [1, half]])
                ss = bass.AP(tensor=sin_c.tensor, offset=sin_c.offset,
                             ap=[sin_c.ap[0], [0, HG], [1, half]])
                t1 = tmp[:].rearrange("p (h d) -> p h d", h=HG)[:, :, 0:half]
                t2 = tmp[:].rearrange("p (h d) -> p h d", h=HG)[:, :, half:D]
                tb = tmp2 if tmp2 is not None else tmp
                t3 = tb[:].rearrange("p (h d) -> p h d", h=HG)[:, :, 0:half]
                t4 = tb[:].rearrange("p (h d) -> p h d", h=HG)[:, :, half:D]
                ed.tensor_tensor(out=t1, in0=se, in1=cc, op=Alu.mult)
                ed.tensor_tensor(out=t2, in0=so, in1=ss, op=Alu.mult)
                ed.tensor_tensor(out=de, in0=t1, in1=t2, op=Alu.subtract)
                eo.tensor_tensor(out=t3, in0=se, in1=ss, op=Alu.mult)
                eo.tensor_tensor(out=t4, in0=so, in1=cc, op=Alu.mult)
                eo.tensor_tensor(out=do, in0=t3, in1=t4, op=Alu.add)

            po_prev = None  # (yt8, ytl) of previous chunk

            def emit_po(ysrcs, c_out, orange=None, final=False):
                yt8_, ytl_ = ysrcs
                nmm = 2 * len(PO_SLOTS)
                for o in (orange if orange is not None else range(NO)):
                    # in the epilogue the projection banks are idle: alternate
                    # pools so o-tile groups overlap their readback copies
                    if final and o % 2 == 1:
                        out_ps = pj.tile([C, GW], F32, tag="big")
                    else:
                        out_ps = ppo.tile([C, GW], F32, tag="po")
                    n = 0
                    # hp-outer: the first MMs only need heads 0-1, so the
                    # group can start before heads 2-3 finish
                    for hp in range(HG // 2):
                        for slot, ysel in PO_SLOTS:
                            ysrc = yt8_ if ysel == 0 else ytl_
                            nc.tensor.matmul(
                                out_ps[:],
                                pair(ysrc, hp * 2 * C, C, C),
                                pair(wo_t,
                                     slot * HG * HID + (2 * hp) * HID + o * GW,
                                     HID, GW),
                                start=(n == 0), stop=(n == nmm - 1),
                                perf_mode=DRm)
                            n += 1
                    out_sb = osb.tile([C, GW], BF16, tag="osb")
                    if o % 2 == 0:
                        nc.scalar.mul(out_sb[:], out_ps[:], 1.0 / SCALE)
                    else:
                        nc.vector.tensor_scalar_mul(out_sb[:], out_ps[:],
                                                    1.0 / SCALE)
                    nc.sync.dma_start(out_d[bass.ts(c_out, C), bass.ts(o, GW)],
                                      out_sb[:])

            def proj_phase(c):
                """Projections + beta chain + rope/phi/khat for chunk c.
                Emitted two chunks ahead of the scan so the elementwise
                chains overlap earlier chunks' scans."""
                if c < len(xpre):
                    xtb = xpre[c]
                    cos_c, sin_c = cspre[c]
                else:
                    xtb = xp.tile([C, 2 * HID], F8, tag="xtb")
                    fetch_x(xtb, c)
                    cos_c, sin_c = fetch_cs(c)

                # ---- projections (PE) interleaved with beta chain ----
                # g first so the long beta dependency chain starts early
                g_ps = pj.tile([C, GW], F32, tag="big")
                proj_mms(g_ps, HG, wg_t, NK * HG, WTERMS["g"], xtb)
                beta_sb = sml.tile([C, HG], F32, tag="beta")
                nc.scalar.activation(beta_sb[:], g_ps[:, 0:HG], Act.Exp,
                                     scale=-1.0 / SCALE)
                nc.vector.scalar_tensor_tensor(
                    out=beta_sb[:], in0=beta_sb[:], scalar=1.0,
                    in1=nbg_t[:], op0=Alu.mult, op1=Alu.mult)
                nc.vector.tensor_scalar_add(beta_sb[:], beta_sb[:], 1.0)
                nc.vector.reciprocal(beta_sb[:], beta_sb[:])
                nc.gpsimd.tensor_scalar(out=beta_sb[:], in0=beta_sb[:],
                                        scalar1=BETA_MIN, scalar2=BETA_MAX,
                                        op0=Alu.max, op1=Alu.min)

                q_ps = pj.tile([C, GW], F32, tag="big")
                proj_mms(q_ps, GW, wq_t, TW, WTERMS["q"], xtb)
                q_sb = big2.tile([C, GW], BF16, tag="q")
                nc.scalar.copy(q_sb[:], q_ps[:])

                # beta transposes ride the pnu rotation between head uses
                btp_ps = pbeta.tile([C, C], F32, tag="bt")
                nc.tensor.transpose(btp_ps[0:HG, 0:C], beta_sb[:], id_t[:])
                btp_sb = sml.tile([HG, C], F32, tag="btp")
                nc.scalar.copy(btp_sb[:], btp_ps[0:HG, 0:C])
                aT_sb = sml.tile([HG, C], F32, tag="aT")
                nc.vector.tensor_tensor_scan(
                    out=aT_sb[:], data0=btp_sb[:], data1=ones_t[0:HG, :],
                    initial=1.0, op0=Alu.mult, op1=Alu.mult)

                k_ps = pj.tile([C, GW], F32, tag="big")
                proj_mms(k_ps, GW, wk_t, TW, WTERMS["k"], xtb)
                k_sb = big2.tile([C, GW], BF16, tag="k")
                nc.scalar.copy(k_sb[:], k_ps[:])

                a_ps = pbeta.tile([C, C], F32, tag="bt")
                nc.tensor.transpose(a_ps[:, 0:HG], aT_sb[:], id_t[0:HG, 0:HG])
                a_sb = sml.tile([C, HG], F32, tag="a")
                nc.scalar.copy(a_sb[:], a_ps[:, 0:HG])
                ainv_sb = sml.tile([C, HG], F32, tag="ainv")
                nc.vector.reciprocal(ainv_sb[:], a_sb[:])
                diag4 = sml.tile([HG, HG], F32, tag="diag4")
                nc.vector.tensor_scalar(out=diag4[:], in0=id_t[0:HG, 0:HG],
                                        scalar1=aT_sb[:, C - 1:C], scalar2=None,
                                        op0=Alu.mult)

                # rope(q) early on DVE; phi(q) min + assembly on GPSIMD
                # (the q path is off the serial S/z recurrence, so the slow
                # Pool engine can carry it)
                qr = big2.tile([C, GW], BF16, tag="qr")
                rtq = big2.tile([C, GW], BF16, tag="rtq")
                rtq2 = big2.tile([C, GW], BF16, tag="rtq2")
                rope(q_sb, qr, rtq, (cos_c, sin_c), rtq2)
                tmq = big2.tile([C, GW], BF16, tag="mq")
                nc.vector.tensor_scalar_min(tmq[:], qr[:], 0.0)
                teq = big2.tile([C, GW], BF16, tag="eq")
                nc.scalar.activation(teq[:], tmq[:], Act.Exp)
                # phi(q) assembled on GPSIMD (no stt opcode there: relu + add)
                rlq = big2.tile([C, GW], BF16, tag="rlq")
                nc.gpsimd.tensor_scalar_max(rlq[:], qr[:], 0.0)
                phiq = big2.tile([C, GW], BF16, tag="phq", bufs=4)
                nc.gpsimd.tensor_tensor(out=phiq[:], in0=rlq[:], in1=teq[:],
                                        op=Alu.add)

                v_ps = pj.tile([C, GW], F32, tag="big")
                proj_mms(v_ps, GW, wv_t, TW, WTERMS["v"], xtb)
                v_sb = big2.tile([C, HG * (D + 1)], BF16, tag="v", bufs=4)
                v_aug = v_sb[:].rearrange("p (h e) -> p h e", e=D + 1)
                nc.scalar.copy(v_aug[:, :, 0:D],
                               v_ps[:, 0:GW].rearrange("p (h e) -> p h e", e=D))
                nc.vector.memset(v_aug[:, :, D:D + 1], SCALE)

                acb_ps = pbeta.tile([C, C], F32, tag="bt")
                nc.tensor.matmul(acb_ps[:, 0:HG], ones_t[0:HG, :], diag4[:],
                                 start=True, stop=True)
                acb_sb = sml.tile([C, HG], F32, tag="acb")
                nc.scalar.copy(acb_sb[:], acb_ps[:, 0:HG])
                acdiv_sb = sml.tile([C, HG], F32, tag="acdiv")
                nc.vector.tensor_tensor(out=acdiv_sb[:], in0=ainv_sb[:],
                                        in1=acb_sb[:], op=Alu.mult)

                # rope(k) + phi(k) stay on the fast engines: the k-path feeds
                # the serial S/z recurrence (khat -> U -> S_new)
                kr = big2.tile([C, GW], BF16, tag="kr")
                rtk = big2.tile([C, GW], BF16, tag="rtk")
                rope(k_sb, kr, rtk, (cos_c, sin_c))
                tmk = big2.tile([C, GW], BF16, tag="mk")
                nc.vector.tensor_scalar_min(tmk[:], kr[:], 0.0)
                tek = big2.tile([C, GW], BF16, tag="ek")
                nc.scalar.activation(tek[:], tmk[:], Act.Exp)
                phik = big2.tile([C, GW], BF16, tag="phk", bufs=4)
                nc.vector.scalar_tensor_tensor(out=phik[:], in0=kr[:],
                                               scalar=0.0, in1=tek[:],
                                               op0=Alu.max, op1=Alu.add)

                # khat for all heads, hoisted off the per-head critical path
                khats = []
                for h in range(HG):
                    khat = sml.tile([C, D], BF16, tag="khat", bufs=12)
                    eng = nc.gpsimd if h < 3 else nc.vector
                    eng.tensor_scalar_mul(khat[:], phik[:, bass.ts(h, D)],
                                          acdiv_sb[:, h:h + 1])
                    khats.append(khat[:])

                return dict(phiq=phiq, phik=phik, khats=khats, v_sb=v_sb,
                            ainv_sb=ainv_sb, acb_sb=acb_sb)

            def scan_phase(c, P, last=False):
                nonlocal po_prev
                phiq, phik = P["phiq"], P["phik"]
                khats, v_sb = P["khats"], P["v_sb"]
                ainv_sb, acb_sb = P["ainv_sb"], P["acb_sb"]

                # ---- scan, 2-wide head pipeline ----
                use_ytl = any(ysel == 1 for _, ysel in PO_SLOTS)
                yt8 = ypool.tile([C, HG * C], F8, tag="yt8")
                if use_ytl:
                    ytl = ypool.tile([C, HG * C], F8, tag="ytl")
                else:
                    ytl = None

                def pair_tp(p):
                    # both heads' q/k transposes land in one PSUM tile so a
                    # single wide DVE copy moves them to SBUF (2x bf16 mode)
                    tp6 = ptp.tile([C, 6 * D], BF16, tag="tp6")
                    for i, h in enumerate((p, p + 1)):
                        hs = bass.ts(h, D)
                        nc.tensor.transpose(tp6[:, (2 * i) * D:(2 * i + 1) * D],
                                            phiq[:, hs], id_s[:])
                        nc.tensor.transpose(tp6[:, (2 * i + 1) * D:(2 * i + 2) * D],
                                            phik[:, hs], id_s[:])
                    qkT = sml.tile([C, 4 * D], BF16, tag="qkT")
                    nc.vector.tensor_copy(qkT[:], tp6[:, 0:4 * D])
                    return tp6, qkT

                def head_A(h, i, st):
                    tp6, qkT = st
                    nuA = pnu.tile([C, 3 * (D + 1) - 1], F32, tag="nuA")
                    Ar = nuA[:, 2 * (D + 1):3 * (D + 1) - 1]
                    nc.tensor.matmul(Ar[:], qkT[:, (2 * i + 1) * D:(2 * i + 2) * D],
                                     qkT[:, (2 * i) * D:(2 * i + 1) * D],
                                     start=True, stop=True)
                    A_sb = sml.tile([C, C], BF16, tag="A")
                    nc.vector.scalar_tensor_tensor(
                        out=A_sb[:], in0=Ar[:],
                        scalar=ainv_sb[:, h:h + 1], in1=mask_t[:],
                        op0=Alu.mult, op1=Alu.mult)
                    return (*st, nuA, A_sb)

                def head_nu(h, i, st):
                    tp6, qkT, nuA, A_sb = st
                    nu = nuA[:, 0:D + 1]
                    U = nuA[:, D + 1:2 * (D + 1)]
                    vh = v_sb[:, h * (D + 1):(h + 1) * (D + 1)]
                    # the A_sb-consuming matmul opens the bank's accumulation
                    # group: it naturally orders after the DVE mask, so the
                    # Ar readback completes before the bank is re-zeroed
                    nc.tensor.matmul(nu[:], A_sb[:], vh, start=True, stop=False)
                    nc.tensor.matmul(U[:], khats[h], vh, start=False,
                                     stop=False)
                    nc.tensor.matmul(nu[:], qkT[:, (2 * i) * D:(2 * i + 1) * D],
                                     S_cur[h][:], start=False, stop=True)
                    # recip/ybf first: the y-transpose unblocks before the
                    # (slack-tolerant) S update; denom = phi_q . z is strictly
                    # positive (phi > 0) so the reference's +eps (~1e-8
                    # relative) is dropped and the reciprocal reads PSUM
                    rd = sml.tile([C, 1], F32, tag="rd")
                    nc.vector.reciprocal(rd[:], nu[:, D:D + 1])
                    y_bf = sml.tile([C, D], BF16, tag="ybf")
                    nc.scalar.activation(y_bf[:], nu[:, 0:D], Act.Copy,
                                         scale=rd[:])
                    S_new = spool.tile([C, D + 1], BF16, tag=f"s{h}")
                    nc.vector.scalar_tensor_tensor(
                        out=S_new[:], in0=S_cur[h][:], scalar=acb_sb[:, h:h + 1],
                        in1=U, op0=Alu.mult, op1=Alu.add)
                    S_cur[h] = S_new
                    return (*st, y_bf)

                def head_yT(h, i, st):
                    tp6, y_bf = st[0], st[-1]
                    nc.tensor.transpose(tp6[:, (4 + i) * D:(5 + i) * D],
                                        y_bf[:], id_s[:])

                # out-projection of the PREVIOUS chunk is interleaved into the
                # scan as PE filler work behind the DVE/ACT dependency chains
                po_open = []
                for p in (0, 2):
                    st = pair_tp(p)
                    if last and p == 2:
                        # last chunk: open its own po groups with the hp=0
                        # MMs — pair 0's yT/ytl exist by now (emitted above);
                        # the groups close after pair 1 lands. pj banks are
                        # idle in the drain (no further projections).
                        for o in (0, 1):
                            out_ps = pj.tile([C, GW], F32, tag="big")
                            n = 0
                            for slot, ysel in PO_SLOTS:
                                ysrc = yt8 if ysel == 0 else ytl
                                nc.tensor.matmul(
                                    out_ps[:], pair(ysrc, 0, C, C),
                                    pair(wo_t, slot * HG * HID + o * GW,
                                         HID, GW),
                                    start=(n == 0), stop=False, perf_mode=DRm)
                                n += 1
                            po_open.append(out_ps)
                    s0 = head_A(p, 0, st)
                    s1 = head_A(p + 1, 1, st)
                    s0 = head_nu(p, 0, s0)
                    s1 = head_nu(p + 1, 1, s1)
                    if po_prev is not None:
                        emit_po(po_prev[0], po_prev[1],
                                orange=(0, 1, 2) if p == 0 else (3,))
                    head_yT(p, 0, s0)
                    head_yT(p + 1, 1, s1)
                    tp6 = st[0]
                    ys = slice(p * C, (p + 2) * C)
                    nc.scalar.copy(yt8[:, ys], tp6[:, 4 * D:6 * D])
                    if use_ytl:
                        nc.vector.tensor_tensor(out=ytl[:, ys],
                                                in0=tp6[:, 4 * D:6 * D],
                                                in1=yt8[:, ys], op=Alu.subtract)
                for o, out_ps in enumerate(po_open):
                    # close the split-opened groups with the hp=1 MMs (pair
                    # 1's yT/ytl were copied just above) and drain them
                    n = 0
                    for slot, ysel in PO_SLOTS:
                        ysrc = yt8 if ysel == 0 else ytl
                        nc.tensor.matmul(
                            out_ps[:], pair(ysrc, 2 * C, C, C),
                            pair(wo_t, slot * HG * HID + 2 * HID + o * GW,
                                 HID, GW),
                            start=False, stop=(n == len(PO_SLOTS) - 1),
                            perf_mode=DRm)
                        n += 1
                    out_sb = osb.tile([C, GW], BF16, tag="osb")
                    nc.scalar.mul(out_sb[:], out_ps[:], 1.0 / SCALE)
                    nc.sync.dma_start(out_d[bass.ts(c, C), bass.ts(o, GW)],
                                      out_sb[:])
                po_prev = ((yt8, ytl), c)

            # ---- chunk pipeline: projections run two chunks ahead of the
            # scan so their elementwise chains overlap earlier scans ----
            pend = [proj_phase(0)]
            if nch > 1:
                pend.append(proj_phase(1))
            for c in range(nch):
                scan_phase(c, pend.pop(0), last=(c == nch - 1))
                if c + 2 < nch:
                    pend.append(proj_phase(c + 2))

            emit_po(po_prev[0], po_prev[1], orange=(2, 3), final=True)

    nc.compile()
    return nc


def _get_nc(cfg_key="default", **cfg):
    if cfg_key not in _CACHE:
        _CACHE[cfg_key] = _build(cfg)
    return _CACHE[cfg_key]


def _blk(m, fw):
    # [HID, fw] -> [C, NK*fw] with block k = m[k*128:(k+1)*128, :]
    return np.ascontiguousarray(
        m.reshape(NK, C, fw).transpose(1, 0, 2).reshape(C, NK * fw))


def _w3(W, fw, nterm):
    """W [HID, fw] f32 -> [C, nterm*NK*fw] e4m3: fp8(32W) | fp8(2W) | fp8(32Wl)."""
    W = np.asarray(W, np.float32)
    t0 = (SCALE * W).astype(E4)
    terms = [_blk(t0, fw), _blk((2.0 * W).astype(E4), fw)]
    if nterm == 3:
        wl = W - t0.astype(np.float32) / SCALE
        terms.append(_blk((SCALE * wl).astype(E4), fw))
    return np.ascontiguousarray(np.concatenate(terms[:nterm], axis=1))


def _wo3(Wo):
    """Wo [GW, HID] f32 -> [C, 2*HG*HID] e4m3, blocked by head: fp8(32Wo)
    (shared by the y8 and y-residual terms, both at 32x scale) | fp8(32*Wol)."""
    Wo = np.asarray(Wo, np.float32)
    t0 = (SCALE * Wo).astype(E4)
    wl = Wo - t0.astype(np.float32) / SCALE
    t2 = (SCALE * wl).astype(E4)

    def blk(m):
        return m.reshape(HG, C, HID).transpose(1, 0, 2).reshape(C, HG * HID)

    return np.ascontiguousarray(np.concatenate([blk(t0), blk(t2)], axis=1))


def make_in_maps(x, Wq, Wk, Wv, Wg, bg, Wo, bo):
    cosr, sinr = _rope_tables()
    maskT = np.triu(np.ones((C, C), np.float32))
    ident = np.eye(C, dtype=np.float32)
    x = np.asarray(x, np.float32)
    Wq, Wk, Wv = np.asarray(Wq), np.asarray(Wk), np.asarray(Wv)
    Wg, bg, Wo = np.asarray(Wg), np.asarray(bg), np.asarray(Wo)
    in_maps = []
    xTb_cache = {}
    for core in range(NCORES):
        b, hg = divmod(core, 4)
        cs = slice(hg * GW, (hg + 1) * GW)
        hsl = slice(hg * HG, (hg + 1) * HG)
        if b not in xTb_cache:
            # xTb[c, p, k*128+f] = x[b][c*128+f, k*128+p]; fp8 + 16*residual
            xT = np.ascontiguousarray(
                x[b].reshape(NCHUNK, C, NK, C).transpose(0, 3, 2, 1)
                .reshape(NCHUNK, C, HID)).astype(np.float32)
            x8 = xT.astype(E4)
            xl8 = (XL_S * (xT - x8.astype(np.float32))).astype(E4)
            xTb_cache[b] = np.ascontiguousarray(
                np.concatenate([x8, xl8], axis=2))
        in_maps.append({
            "xTb": xTb_cache[b],
            "wq3": _w3(Wq[:, cs], GW, WTERMS["q"]),
            "wk3": _w3(Wk[:, cs], GW, WTERMS["k"]),
            "wv3": _w3(Wv[:, cs], GW, WTERMS["v"]),
            "wg3": _w3(Wg[:, hsl], HG, WTERMS["g"]),
            "wo3": _wo3(Wo[cs, :]),
            "nbg4": np.tile(np.exp(-bg[None, hsl]), (C, 1)).astype(np.float32),
            "cosr": cosr, "sinr": sinr,
            "maskT": maskT, "ident": ident,
        })
    return in_maps


def kernel(x, Wq, Wk, Wv, Wg, bg, Wo, bo, _trace=False, **cfg):
    from concourse.bass_utils import run_bass_kernel_spmd
    nc = _get_nc(**cfg)
    in_maps = make_in_maps(x, Wq, Wk, Wv, Wg, bg, Wo, bo)
    res = run_bass_kernel_spmd(nc, in_maps, core_ids=list(range(NCORES)),
                               trace=_trace)
    out = np.zeros((B, L, HID), np.float32)
    for core in range(NCORES):
        b = core // 4
        out[b] += res.results[core]["out"].astype(np.float32)
    out += np.asarray(bo, np.float32)[None, None, :]
    kernel._last_results = res
    return out

